# revision 30
# baseline (speedup 1.0000x reference)
"""Trainium2 Bass kernel for a 2-bit-quantized (DoReFa) ResNet BasicBlock.

Full (unsharded) numpy inputs -> full numpy output.

Design:
  - batch (64) is sharded 8 images/core across 8 NeuronCores (pure data
    parallel, weights/BN params replicated).
  - 2-bit quantization makes every conv input an exact small integer:
    acts*3 in {0..3} (or sign-coded 2*qa-3), weights*3 in {-3,-1,1,3}.
    Those are exact in fp8e4 and the PE accumulates in fp32, so both
    convs are bit-exact integer matmuls at fp8 DoubleRow speed. All
    scale factors (the /9, BN scale/shift) fold into per-channel
    epilogue constants on host.
  - each 3x3 conv = 9 shifted matmuls accumulated in PSUM over padded
    30-wide activation planes; the moving operand is a flat 418-element
    run across 14 plane rows. Matmuls are emitted tap-major across the
    two h-halves of an output-channel group so consecutive matmuls share
    one stationary lhsT: the PE's background weight buffer then hides
    the 162ns DoubleRow LDWEIGHTS behind 2x177ns of streaming, which is
    the difference between a 212ns and a 177ns matmul cadence.
  - each (image, co) conv uses one 2-bank-aligned psum tile [128,2,512]
    (h-half per bank) so the whole epilogue runs as FD=784 ops, halving
    the per-instruction fixed cost.
  - epilogues write contiguous staging tiles; a single DMA per image
    scatters them into the padded planes (strided writes cost the
    engines nothing).
  - conv1 epilogue: BN+ReLU+requant folds into 3 per-channel psum-space
    thresholds; qa2 = chained is_gt ops (tensor_scalar + 2 fused
    scalar_tensor_tensor) on DVE. conv2 epilogue: v = ps*s2 + x (fused
    DVE op), out = relu(v + bb2) (ACT bias), one store per (image, co).
  - x-quant: image 0 on DVE (is_gt chain, prologue critical path);
    other images on ScalarE (3 ACT Signs -> S = 2*qa-3, pad -3) with
    the two adds on GpSimd, using sign-space conv1 thresholds t1s.
  - a scratch-fed run of dummy DoubleRow matmuls at kernel start warms
    the PE HAM clock gate and covers the x-load + quantize prologue so
    the real matmul stream starts warm and never throttles.
  - weight quantization (tanh / global max / round) + BN folding is done
    on host: it is O(weights) = 0.6 MB, vs 118 GFLOP of conv on device.
"""

import os
import sys
import numpy as np


def _install_ntff_hook_shim():
    """Provide antenv.axon_hooks if the image lacks it, so
    run_bass_kernel_spmd(trace=True) can capture NTFF profiles through
    libaxon_pjrt.so. No-op if the real module exists or the .so is absent."""
    try:
        import antenv.axon_hooks  # noqa: F401
        return
    except ImportError:
        pass
    import contextlib
    import ctypes
    import types

    so_path = "/opt/axon/libaxon_pjrt.so"
    _hook = None
    if os.path.exists(so_path):
        try:
            lib = ctypes.CDLL(so_path)
        except OSError:
            lib = None
        if lib is not None and hasattr(lib, "axon_start_nrt_profile"):
            lib.axon_start_nrt_profile.argtypes = [
                ctypes.POINTER(ctypes.c_int64), ctypes.c_size_t]
            lib.axon_start_nrt_profile.restype = ctypes.c_int64
            lib.axon_stop_nrt_profile.argtypes = [ctypes.c_char_p]
            lib.axon_stop_nrt_profile.restype = ctypes.c_int64

            @contextlib.contextmanager
            def _hook(output_dir, device_ids):
                import jax
                jax.devices()
                if device_ids:
                    ids = (ctypes.c_int64 * len(device_ids))(*device_ids)
                    rc = lib.axon_start_nrt_profile(ids, len(device_ids))
                else:
                    rc = lib.axon_start_nrt_profile(None, 0)
                if rc != 0:
                    raise RuntimeError(f"axon_start_nrt_profile rc={rc}")
                try:
                    yield
                finally:
                    n = lib.axon_stop_nrt_profile(str(output_dir).encode())
                    print(f"profile: {n} file(s) written to {output_dir}",
                          file=sys.stderr)

    mod = types.ModuleType("antenv.axon_hooks")
    mod.get_axon_ntff_profile_hook = lambda: _hook
    mod.set_axon_ntff_profile_hook = lambda h: None
    sys.modules["antenv.axon_hooks"] = mod


NCORES = 8
NPER = 8          # images per core
C = 256
NCH = 2           # channel chunks of 128
H = W = 28
PH = H + 2        # padded plane 30x30
PW = 30           # plane row stride
QSTR = 960        # allocated plane stride (16B-aligned, >= PH*PW)
HALF = 14         # rows per psum half-tile
RUN = (HALF - 1) * PW + W   # 418-element flat moving-run per matmul
PSH = 512         # psum half stride (one full bank of fp32)
BN_EPS = 1e-5
NWARM = int(os.environ.get("KERNEL_NWARM", "20"))   # PE warm-up matmuls
XQV = int(os.environ.get("KERNEL_XQV", "2"))        # imgs with x-quant on DVE


def _quant_weight3(w):
    """Replicate reference _quant_weight in f32, scaled by 3 -> {-3,-1,1,3}."""
    w = np.asarray(w, np.float32)
    t = np.tanh(w)
    m = np.max(np.abs(t))
    t2 = t / (np.float32(2.0) * m) + np.float32(0.5)
    k = np.round(t2 * np.float32(3.0))          # round-half-even == jnp.round
    return (2.0 * k - 3.0).astype(np.float32)


def _fold_bn(g, b, m, v):
    inv = np.asarray(g, np.float64) / np.sqrt(np.asarray(v, np.float64) + BN_EPS)
    beta = np.asarray(b, np.float64) - np.asarray(m, np.float64) * inv
    return inv, beta


def _w_tiles(qw3, dt):
    # [O, I, 3, 3] -> [p=128, ci=2, k=9, O=256] so lhsT slices are
    # [128, 2, 128] interleaved chunks (fp8 DoubleRow).
    return np.ascontiguousarray(
        np.transpose(qw3.reshape(C, NCH, 128, 9), (2, 1, 3, 0))
    ).astype(dt)


def _host_arrays(w1, g1, b1, m1, v1, w2, g2, b2, m2, v2):
    from concourse import mybir
    qw3_1 = _quant_weight3(w1)
    qw3_2 = _quant_weight3(w2)
    inv1, beta1 = _fold_bn(g1, b1, m1, v1)
    inv2, beta2 = _fold_bn(g2, b2, m2, v2)

    act_np = mybir.dt.np(mybir.dt.float8e4)
    w1t = _w_tiles(qw3_1, act_np)
    w2t = _w_tiles(qw3_2, act_np)

    # conv1 psum P1 = 9*conv_true (exact int); y = P1*inv1/9 + beta1.
    # quant level k iff y > tau_k = (2k-1)/6, i.e. P1 > (tau_k-beta1)*9/inv1
    # (inv1 > 0 given g1=1, v1 > 0), so qa2 = sum_k is_gt(P1, t1_k).
    assert np.all(inv1 > 0), "bn1 scale must be positive for threshold fold"
    taus = np.array([1.0, 3.0, 5.0]) / 6.0
    t1 = ((taus[None, :] - beta1[:, None]) * 9.0 / inv1[:, None])  # [C, 3]
    # sign-coded images store S1 = 2*qa1-3 (pad -3), so P1s = 2*P1 - 3*K1f
    # and the thresholds become 2*t1 - 3*K1f per output channel.
    k1f = qw3_1.reshape(C, -1).sum(axis=1)[:, None]                # [C, 1]
    t1s = 2.0 * t1 - 3.0 * k1f

    def fold_t(t):
        return np.ascontiguousarray(
            t.reshape(NCH, 128, 3).transpose(1, 0, 2)).astype(np.float32)

    # conv2 on qa2 in {0..3} with zero padding: P2 = 9*conv2_true, so
    # y2 = P2*(inv2/9) + beta2 and out = relu(y2 + x).
    s2 = np.ascontiguousarray(
        (inv2 / 9.0).reshape(NCH, 128).T).astype(np.float32)
    bb2 = np.ascontiguousarray(
        beta2.reshape(NCH, 128).T).astype(np.float32)
    z0 = np.zeros((128, NCH, QSTR), act_np)
    zm3 = np.full((128, NCH, QSTR), -3.0, act_np)
    xqtn = np.broadcast_to(
        -np.array([1.0, 3.0, 5.0], np.float32) / 6.0, (128, 3)).copy()
    return {"w1t": w1t, "w2t": w2t, "t1": fold_t(t1), "t1s": fold_t(t1s),
            "s2": s2, "bb2": bb2, "z0": z0, "zm3": zm3, "xqtn": xqtn}


def _build_program(nper=NPER):
    from concourse import bacc, tile, mybir
    dt = mybir.dt
    dt_act = dt.float8e4
    ALU = mybir.AluOpType

    nc = bacc.Bacc("TRN2", target_bir_lowering=False, debug=False,
                   num_devices=NCORES)
    NP_ = nper

    x_d = nc.dram_tensor("x", [NP_, C, H, W], dt.float32, kind="ExternalInput")
    w1_d = nc.dram_tensor("w1t", [128, NCH, 9, C], dt_act, kind="ExternalInput")
    w2_d = nc.dram_tensor("w2t", [128, NCH, 9, C], dt_act, kind="ExternalInput")
    t1_d = nc.dram_tensor("t1", [128, NCH, 3], dt.float32, kind="ExternalInput")
    t1s_d = nc.dram_tensor("t1s", [128, NCH, 3], dt.float32,
                           kind="ExternalInput")
    s2_d = nc.dram_tensor("s2", [128, NCH], dt.float32, kind="ExternalInput")
    b2_d = nc.dram_tensor("bb2", [128, NCH], dt.float32, kind="ExternalInput")
    z0_d = nc.dram_tensor("z0", [128, NCH, QSTR], dt_act, kind="ExternalInput")
    zm3_d = nc.dram_tensor("zm3", [128, NCH, QSTR], dt_act,
                           kind="ExternalInput")
    xqtn_d = nc.dram_tensor("xqtn", [128, 3], dt.float32,
                            kind="ExternalInput")
    y_d = nc.dram_tensor("y", [NP_, C, H, W], dt.float32, kind="ExternalOutput")

    XQT = [1.0 / 6.0, 3.0 / 6.0, 5.0 / 6.0]   # act-quant thresholds for x

    with tile.TileContext(nc) as tc:
        with (
            tc.tile_pool(name="wpool", bufs=1) as wpool,
            tc.tile_pool(name="xpool", bufs=2 * NP_) as xpool,
            tc.tile_pool(name="qpool", bufs=NP_) as qpool,
            tc.tile_pool(name="spool", bufs=3) as spool,
            tc.tile_pool(name="mpool", bufs=4) as mpool,
            tc.tile_pool(name="upool", bufs=4) as upool,
            tc.tile_pool(name="opool", bufs=4) as opool,
            tc.tile_pool(name="pspool", bufs=4, space="PSUM") as pspool,
        ):
            w1_sb = wpool.tile([128, NCH, 9, C], dt_act, name="w1sb")
            w2_sb = wpool.tile([128, NCH, 9, C], dt_act, name="w2sb")
            t1_sb = wpool.tile([128, NCH, 3], dt.float32, name="t1sb")
            t1s_sb = wpool.tile([128, NCH, 3], dt.float32, name="t1ssb")
            xqtn_sb = wpool.tile([128, 3], dt.float32, name="xqtnsb")
            s2_sb = wpool.tile([128, NCH], dt.float32, name="s2sb")
            b2_sb = wpool.tile([128, NCH], dt.float32, name="b2sb")
            wscr = wpool.tile([128, NCH, 512], dt_act, name="wscr")
            # zero/-3 padded quantized-activation planes (flat, per image)
            qa1 = [qpool.tile([128, NCH, QSTR], dt_act, name=f"qa1_{n}",
                              tag="qa1") for n in range(NP_)]
            qa2 = [qpool.tile([128, NCH, QSTR], dt_act, name=f"qa2_{n}",
                              tag="qa2") for n in range(NP_)]

            def plane_interior(qa_t, j):
                # [128, 28, 28] view of chunk j's (1..28, 1..28) interior
                return qa_t[:, j, 31:31 + H * PW].rearrange(
                    "p (r c) -> p r c", c=PW)[:, :, 0:W]

            x_sb = [[None] * NCH for _ in range(NP_)]

            def load_x(n):
                for j in range(NCH):
                    xt = xpool.tile([128, H, W], dt.float32,
                                    name=f"x_{n}_{j}", tag="x")
                    nc.sync.dma_start(xt[:],
                                      x_d[n, j * 128:(j + 1) * 128, :, :])
                    x_sb[n][j] = xt

            def fill_planes(n):
                sign_coded = n >= XQV
                nc.gpsimd.dma_start(qa1[n][:],
                                    zm3_d[:] if sign_coded else z0_d[:])
                nc.gpsimd.dma_start(qa2[n][:], z0_d[:])

            # --- prologue: scratch memset, image-0 x, conv1 weights, then
            # PE warm-up matmuls that cover the x-quant latency.
            nc.gpsimd.memset(wscr[:], 0)
            load_x(0)
            fill_planes(0)
            nc.sync.dma_start(w1_sb[:], w1_d[:])
            for i in range(NWARM):
                wps = pspool.tile([128, NCH, PSH], dt.float32, name="wps",
                                  tag="ps")
                nc.tensor.matmul(
                    wps[:, 0, 0:RUN],
                    wscr[:, 0:NCH, 0:128],
                    wscr[:, 0:NCH, 0:RUN],
                    start=True, stop=True,
                    perf_mode=mybir.MatmulPerfMode.DoubleRow,
                )

            # remaining params on the gpsimd queue; x loads own sync
            nc.gpsimd.dma_start(w2_sb[:], w2_d[:])
            nc.gpsimd.dma_start(t1_sb[:], t1_d[:])
            nc.gpsimd.dma_start(t1s_sb[:], t1s_d[:])
            nc.gpsimd.dma_start(xqtn_sb[:], xqtn_d[:])
            nc.gpsimd.dma_start(s2_sb[:], s2_d[:])
            nc.gpsimd.dma_start(b2_sb[:], b2_d[:])

            def xq_image(n):
                # quantize x into a contiguous staging tile, then one DMA
                # scatters both chunks into the padded qa1 planes.
                if n > 0:
                    load_x(n)
                    fill_planes(n)
                qs1 = spool.tile([128, NCH, H, W], dt_act, name="qs1",
                                 tag="qs1")
                for j in range(NCH):
                    xt = x_sb[n][j]
                    if n < XQV:
                        m1 = mpool.tile([128, H, W], dt.bfloat16, name="m1",
                                        tag="xv1")
                        m2 = mpool.tile([128, H, W], dt.bfloat16, name="m2",
                                        tag="xv2")
                        nc.vector.tensor_scalar(
                            m1[:], xt[:], XQT[0], None, ALU.is_gt)
                        nc.vector.scalar_tensor_tensor(
                            m2[:], xt[:], XQT[1], m1[:], ALU.is_gt, ALU.add)
                        nc.vector.scalar_tensor_tensor(
                            qs1[:, j], xt[:], XQT[2], m2[:],
                            ALU.is_gt, ALU.add)
                    else:
                        s1 = mpool.tile([128, H, W], dt.bfloat16, name="s1",
                                        tag="xg1")
                        s2m = mpool.tile([128, H, W], dt.bfloat16, name="s2m",
                                         tag="xg2")
                        s3 = mpool.tile([128, H, W], dt.bfloat16, name="s3",
                                        tag="xg3")
                        st = mpool.tile([128, H, W], dt.bfloat16, name="st",
                                        tag="xg4")
                        for k, sk in enumerate((s1, s2m, s3)):
                            nc.scalar.activation(
                                sk[:], xt[:],
                                mybir.ActivationFunctionType.Sign,
                                bias=xqtn_sb[:, k:k + 1])
                        nc.gpsimd.tensor_tensor(st[:], s1[:], s2m[:], ALU.add)
                        nc.gpsimd.tensor_tensor(qs1[:, j], st[:], s3[:],
                                                ALU.add)
                    nc.gpsimd.dma_start(plane_interior(qa1[n], j), qs1[:, j])

            def conv_mms2(ps, w_sb, qa_n, co):
                # tap-major over both h-halves: consecutive matmuls share
                # one stationary lhsT so the PE's background weight buffer
                # hides the DoubleRow LDWEIGHTS behind the moving stream.
                for k in range(9):
                    dy, dx = divmod(k, 3)
                    for h in range(2):
                        off = (h * HALF + dy) * PW + dx
                        nc.tensor.matmul(
                            ps[:, h, 0:RUN],
                            w_sb[:, 0:NCH, k, co * 128:(co + 1) * 128],
                            qa_n[:, 0:NCH, off:off + RUN],
                            start=(k == 0), stop=(k == 8),
                            perf_mode=mybir.MatmulPerfMode.DoubleRow,
                        )

            def psum_pair(name):
                # one [128, 2, 512] fp32 tile = two aligned PSUM banks;
                # each h-half is a legal single-bank matmul target and the
                # epilogue reads both halves in one 3D FD=840 op (the 2
                # garbage columns per row are computed and then skipped by
                # the plane scatter).
                ps = pspool.tile([128, NCH, PSH], dt.float32, name=name,
                                 tag="ps")
                return ps, ps[:, :, 0:HALF * PW]

            # conv1 -> bn1 -> relu -> quant, folded into 3 per-channel
            # is_gt thresholds accumulated with fused STT ops on DVE
            def conv1_image(n):
                thr = t1s_sb if n >= XQV else t1_sb
                qs2 = spool.tile([128, NCH, NCH, HALF * PW], dt_act,
                                 name="qs2", tag="qs2")
                for co in range(NCH):
                    ps, psv = psum_pair("ps1")
                    conv_mms2(ps, w1_sb, qa1[n], co)
                    e1 = mpool.tile([128, NCH, HALF * PW], dt.bfloat16,
                                    name="e1", tag="e1")
                    e2 = mpool.tile([128, NCH, HALF * PW], dt.bfloat16,
                                    name="e2", tag="e2")
                    nc.vector.tensor_scalar(
                        e1[:], psv, thr[:, co, 0:1], None, ALU.is_gt)
                    nc.vector.scalar_tensor_tensor(
                        e2[:], psv, thr[:, co, 1:2], e1[:], ALU.is_gt,
                        ALU.add)
                    nc.vector.scalar_tensor_tensor(
                        qs2[:, co], psv, thr[:, co, 2:3], e2[:], ALU.is_gt,
                        ALU.add)
                    # interior rows across both h-halves are uniformly
                    # 30-strided, so one 3-dim scatter covers this co.
                    # scalar ring: its other work (relu) is late-ready too,
                    # so this never head-of-line blocks early transfers.
                    nc.scalar.dma_start(
                        plane_interior(qa2[n], co),
                        qs2[:, co].rearrange(
                            "p g (r c) -> p (g r) c", c=PW)[:, :, 0:W])

            # conv2 -> bn2 -> +residual -> relu -> out:
            #   v = ps*s2 + x (DVE STT), out = relu(v + bb2) (ACT bias)
            def conv2_image(n):
                for co in range(NCH):
                    ps, psv = psum_pair("ps2")
                    conv_mms2(ps, w2_sb, qa2[n], co)
                    v = upool.tile([128, H, W], dt.float32, name="v", tag="v")
                    o = opool.tile([128, H, W], dt.float32, name="o", tag="o")
                    for h in range(2):
                        psvh = psv[:, h].rearrange(
                            "p (r c) -> p r c", c=PW)[:, :, 0:W]
                        nc.vector.scalar_tensor_tensor(
                            v[:][:, h * HALF:(h + 1) * HALF, :], psvh,
                            s2_sb[:, co:co + 1],
                            x_sb[n][co][:, h * HALF:(h + 1) * HALF, :],
                            ALU.mult, ALU.add)
                    nc.scalar.activation(
                        o[:], v[:], mybir.ActivationFunctionType.Relu,
                        bias=b2_sb[:, co:co + 1])
                    nc.sync.dma_start(
                        y_d[n, co * 128:(co + 1) * 128, :, :], o[:])

            # software-pipelined emission, x-quant running two images
            # ahead: on the in-order scalar queue the early-ready Sign
            # ACTs of image n+2 are emitted before the late-ready relu of
            # image n-1, so they never wait behind it.
            xq_image(0)
            if NP_ > 1:
                xq_image(1)
            for n in range(NP_):
                conv1_image(n)
                if n + 2 < NP_:
                    xq_image(n + 2)
                if n >= 1:
                    conv2_image(n - 1)
            conv2_image(NP_ - 1)

    nc.compile()
    return nc


_CACHED = None


def _get_program():
    global _CACHED
    if _CACHED is None:
        _CACHED = _build_program()
    return _CACHED


def kernel(x, w1, g1, b1, m1, v1, w2, g2, b2, m2, v2):
    _install_ntff_hook_shim()
    from concourse.bass_utils import run_bass_kernel_spmd

    x = np.asarray(x, np.float32)
    host = _host_arrays(w1, g1, b1, m1, v1, w2, g2, b2, m2, v2)

    xs = x.reshape(NCORES, NPER, C, H, W)
    in_maps = [{"x": np.ascontiguousarray(xs[c]), **host}
               for c in range(NCORES)]

    nc = _get_program()
    res = run_bass_kernel_spmd(
        nc, in_maps, core_ids=list(range(NCORES)),
        trace=bool(int(os.environ.get("KERNEL_TRACE", "0"))),
    )
    kernel.last_results = res
    y = np.concatenate([res.results[c]["y"][None] for c in range(NCORES)], 0)
    return np.ascontiguousarray(y.reshape(64, C, H, W).astype(np.float32))


# revision 36
# speedup vs baseline: 1.0468x; 1.0468x over previous
"""Trainium2 Bass kernel for a 2-bit-quantized (DoReFa) ResNet BasicBlock.

Full (unsharded) numpy inputs -> full numpy output.

Design:
  - batch (64) is sharded 8 images/core across 8 NeuronCores (pure data
    parallel, weights/BN params replicated).
  - 2-bit quantization makes every conv input an exact small integer:
    acts*3 in {0..3} (or sign-coded 2*qa-3), weights*3 in {-3,-1,1,3}.
    Those are exact in fp8e4 and the PE accumulates in fp32, so both
    convs are bit-exact integer matmuls at fp8 DoubleRow speed. All
    scale factors (the /9, BN scale/shift) fold into per-channel
    epilogue constants on host.
  - each 3x3 conv = 9 shifted matmuls accumulated in PSUM over padded
    30-wide activation planes; the moving operand is a flat 418-element
    run across 14 plane rows. Matmuls are emitted tap-major across the
    two h-halves of an output-channel group so consecutive matmuls share
    one stationary lhsT: the PE's background weight buffer then hides
    the 162ns DoubleRow LDWEIGHTS behind 2x177ns of streaming, which is
    the difference between a 212ns and a 177ns matmul cadence.
  - each (image, co) conv uses one 2-bank-aligned psum tile [128,2,512]
    (h-half per bank) so the whole epilogue runs as FD=784 ops, halving
    the per-instruction fixed cost.
  - epilogues write contiguous staging tiles; a single DMA per image
    scatters them into the padded planes (strided writes cost the
    engines nothing).
  - conv1 epilogue: BN+ReLU+requant folds into 3 per-channel psum-space
    thresholds; qa2 = chained is_gt ops (tensor_scalar + 2 fused
    scalar_tensor_tensor) on DVE. conv2 epilogue: v = ps*s2 + x (fused
    DVE op), out = relu(v + bb2) (ACT bias), one store per (image, co).
  - x-quant: image 0 on DVE (is_gt chain, prologue critical path);
    other images on ScalarE (3 ACT Signs -> S = 2*qa-3, pad -3) with
    the two adds on GpSimd, using sign-space conv1 thresholds t1s.
  - a scratch-fed run of dummy DoubleRow matmuls at kernel start warms
    the PE HAM clock gate and covers the x-load + quantize prologue so
    the real matmul stream starts warm and never throttles.
  - weight quantization (tanh / global max / round) + BN folding is done
    on host: it is O(weights) = 0.6 MB, vs 118 GFLOP of conv on device.
"""

import os
import sys
import numpy as np


def _install_ntff_hook_shim():
    """Provide antenv.axon_hooks if the image lacks it, so
    run_bass_kernel_spmd(trace=True) can capture NTFF profiles through
    libaxon_pjrt.so. No-op if the real module exists or the .so is absent."""
    try:
        import antenv.axon_hooks  # noqa: F401
        return
    except ImportError:
        pass
    import contextlib
    import ctypes
    import types

    so_path = "/opt/axon/libaxon_pjrt.so"
    _hook = None
    if os.path.exists(so_path):
        try:
            lib = ctypes.CDLL(so_path)
        except OSError:
            lib = None
        if lib is not None and hasattr(lib, "axon_start_nrt_profile"):
            lib.axon_start_nrt_profile.argtypes = [
                ctypes.POINTER(ctypes.c_int64), ctypes.c_size_t]
            lib.axon_start_nrt_profile.restype = ctypes.c_int64
            lib.axon_stop_nrt_profile.argtypes = [ctypes.c_char_p]
            lib.axon_stop_nrt_profile.restype = ctypes.c_int64

            @contextlib.contextmanager
            def _hook(output_dir, device_ids):
                import jax
                jax.devices()
                if device_ids:
                    ids = (ctypes.c_int64 * len(device_ids))(*device_ids)
                    rc = lib.axon_start_nrt_profile(ids, len(device_ids))
                else:
                    rc = lib.axon_start_nrt_profile(None, 0)
                if rc != 0:
                    raise RuntimeError(f"axon_start_nrt_profile rc={rc}")
                try:
                    yield
                finally:
                    n = lib.axon_stop_nrt_profile(str(output_dir).encode())
                    print(f"profile: {n} file(s) written to {output_dir}",
                          file=sys.stderr)

    mod = types.ModuleType("antenv.axon_hooks")
    mod.get_axon_ntff_profile_hook = lambda: _hook
    mod.set_axon_ntff_profile_hook = lambda h: None
    sys.modules["antenv.axon_hooks"] = mod


NCORES = 8
NPER = 8          # images per core
C = 256
NCH = 2           # channel chunks of 128
H = W = 28
PH = H + 2        # padded plane 30x30
PW = 30           # plane row stride
QSTR = 960        # allocated plane stride (16B-aligned, >= PH*PW)
HALF = 14         # rows per psum half-tile
RUN = (HALF - 1) * PW + W   # 418-element flat moving-run per matmul
PSH = 512         # psum half stride (one full bank of fp32)
BN_EPS = 1e-5
NWARM = int(os.environ.get("KERNEL_NWARM", "48"))   # PE warm-up matmuls
XQV = int(os.environ.get("KERNEL_XQV", "2"))        # imgs with x-quant on DVE


def _quant_weight3(w):
    """Replicate reference _quant_weight in f32, scaled by 3 -> {-3,-1,1,3}."""
    w = np.asarray(w, np.float32)
    t = np.tanh(w)
    m = np.max(np.abs(t))
    t2 = t / (np.float32(2.0) * m) + np.float32(0.5)
    k = np.round(t2 * np.float32(3.0))          # round-half-even == jnp.round
    return (2.0 * k - 3.0).astype(np.float32)


def _fold_bn(g, b, m, v):
    inv = np.asarray(g, np.float64) / np.sqrt(np.asarray(v, np.float64) + BN_EPS)
    beta = np.asarray(b, np.float64) - np.asarray(m, np.float64) * inv
    return inv, beta


def _w_tiles(qw3, dt):
    # [O, I, 3, 3] -> [p=128, ci=2, k=9, O=256] so lhsT slices are
    # [128, 2, 128] interleaved chunks (fp8 DoubleRow).
    return np.ascontiguousarray(
        np.transpose(qw3.reshape(C, NCH, 128, 9), (2, 1, 3, 0))
    ).astype(dt)


def _host_arrays(w1, g1, b1, m1, v1, w2, g2, b2, m2, v2):
    from concourse import mybir
    qw3_1 = _quant_weight3(w1)
    qw3_2 = _quant_weight3(w2)
    inv1, beta1 = _fold_bn(g1, b1, m1, v1)
    inv2, beta2 = _fold_bn(g2, b2, m2, v2)

    act_np = mybir.dt.np(mybir.dt.float8e4)
    w1t = _w_tiles(qw3_1, act_np)
    w2t = _w_tiles(qw3_2, act_np)

    # conv1 psum P1 = 9*conv_true (exact int); y = P1*inv1/9 + beta1.
    # quant level k iff y > tau_k = (2k-1)/6, i.e. P1 > (tau_k-beta1)*9/inv1
    # (inv1 > 0 given g1=1, v1 > 0), so qa2 = sum_k is_gt(P1, t1_k).
    assert np.all(inv1 > 0), "bn1 scale must be positive for threshold fold"
    taus = np.array([1.0, 3.0, 5.0]) / 6.0
    t1 = ((taus[None, :] - beta1[:, None]) * 9.0 / inv1[:, None])  # [C, 3]
    # sign-coded images store S1 = 2*qa1-3 (pad -3), so P1s = 2*P1 - 3*K1f
    # and the thresholds become 2*t1 - 3*K1f per output channel.
    k1f = qw3_1.reshape(C, -1).sum(axis=1)[:, None]                # [C, 1]
    t1s = 2.0 * t1 - 3.0 * k1f

    def fold_t(t):
        return np.ascontiguousarray(
            t.reshape(NCH, 128, 3).transpose(1, 0, 2)).astype(np.float32)

    # conv2 on qa2 in {0..3} with zero padding: P2 = 9*conv2_true, so
    # y2 = P2*(inv2/9) + beta2 and out = relu(y2 + x).
    s2 = np.ascontiguousarray(
        (inv2 / 9.0).reshape(NCH, 128).T).astype(np.float32)
    bb2 = np.ascontiguousarray(
        beta2.reshape(NCH, 128).T).astype(np.float32)
    z0 = np.zeros((128, NCH, QSTR), act_np)
    zm3 = np.full((128, NCH, QSTR), -3.0, act_np)
    xqtn = np.broadcast_to(
        -np.array([1.0, 3.0, 5.0], np.float32) / 6.0, (128, 3)).copy()
    return {"w1t": w1t, "w2t": w2t, "t1": fold_t(t1), "t1s": fold_t(t1s),
            "s2": s2, "bb2": bb2, "z0": z0, "zm3": zm3, "xqtn": xqtn}


def _build_program(nper=NPER):
    from concourse import bacc, tile, mybir
    dt = mybir.dt
    dt_act = dt.float8e4
    ALU = mybir.AluOpType

    nc = bacc.Bacc("TRN2", target_bir_lowering=False, debug=False,
                   num_devices=NCORES)
    NP_ = nper

    x_d = nc.dram_tensor("x", [NP_, C, H, W], dt.float32, kind="ExternalInput")
    w1_d = nc.dram_tensor("w1t", [128, NCH, 9, C], dt_act, kind="ExternalInput")
    w2_d = nc.dram_tensor("w2t", [128, NCH, 9, C], dt_act, kind="ExternalInput")
    t1_d = nc.dram_tensor("t1", [128, NCH, 3], dt.float32, kind="ExternalInput")
    t1s_d = nc.dram_tensor("t1s", [128, NCH, 3], dt.float32,
                           kind="ExternalInput")
    s2_d = nc.dram_tensor("s2", [128, NCH], dt.float32, kind="ExternalInput")
    b2_d = nc.dram_tensor("bb2", [128, NCH], dt.float32, kind="ExternalInput")
    z0_d = nc.dram_tensor("z0", [128, NCH, QSTR], dt_act, kind="ExternalInput")
    zm3_d = nc.dram_tensor("zm3", [128, NCH, QSTR], dt_act,
                           kind="ExternalInput")
    xqtn_d = nc.dram_tensor("xqtn", [128, 3], dt.float32,
                            kind="ExternalInput")
    y_d = nc.dram_tensor("y", [NP_, C, H, W], dt.float32, kind="ExternalOutput")

    XQT = [1.0 / 6.0, 3.0 / 6.0, 5.0 / 6.0]   # act-quant thresholds for x

    with tile.TileContext(nc) as tc:
        with (
            tc.tile_pool(name="wpool", bufs=1) as wpool,
            tc.tile_pool(name="xpool", bufs=2 * NP_) as xpool,
            tc.tile_pool(name="qpool", bufs=NP_) as qpool,
            tc.tile_pool(name="spool", bufs=3) as spool,
            tc.tile_pool(name="mpool", bufs=4) as mpool,
            tc.tile_pool(name="upool", bufs=4) as upool,
            tc.tile_pool(name="opool", bufs=4) as opool,
            tc.tile_pool(name="pspool", bufs=4, space="PSUM") as pspool,
        ):
            w1_sb = wpool.tile([128, NCH, 9, C], dt_act, name="w1sb")
            w2_sb = wpool.tile([128, NCH, 9, C], dt_act, name="w2sb")
            t1_sb = wpool.tile([128, NCH, 3], dt.float32, name="t1sb")
            t1s_sb = wpool.tile([128, NCH, 3], dt.float32, name="t1ssb")
            xqtn_sb = wpool.tile([128, 3], dt.float32, name="xqtnsb")
            s2_sb = wpool.tile([128, NCH], dt.float32, name="s2sb")
            b2_sb = wpool.tile([128, NCH], dt.float32, name="b2sb")
            wscr = wpool.tile([128, NCH, 512], dt_act, name="wscr")
            # zero/-3 padded quantized-activation planes (flat, per image)
            qa1 = [qpool.tile([128, NCH, QSTR], dt_act, name=f"qa1_{n}",
                              tag="qa1") for n in range(NP_)]
            qa2 = [qpool.tile([128, NCH, QSTR], dt_act, name=f"qa2_{n}",
                              tag="qa2") for n in range(NP_)]

            def plane_interior(qa_t, j):
                # [128, 28, 28] view of chunk j's (1..28, 1..28) interior
                return qa_t[:, j, 31:31 + H * PW].rearrange(
                    "p (r c) -> p r c", c=PW)[:, :, 0:W]

            x_sb = [[None] * NCH for _ in range(NP_)]

            def load_x(n):
                for j in range(NCH):
                    xt = xpool.tile([128, H, W], dt.float32,
                                    name=f"x_{n}_{j}", tag="x")
                    nc.sync.dma_start(xt[:],
                                      x_d[n, j * 128:(j + 1) * 128, :, :])
                    x_sb[n][j] = xt

            def fill_planes(n):
                # all DMAs ride the sync HW-DGE ring: the gpsimd/scalar
                # SWDGE rings have 10us-class issue-to-completion latency
                sign_coded = n >= XQV
                nc.sync.dma_start(qa1[n][:],
                                  zm3_d[:] if sign_coded else z0_d[:])
                nc.sync.dma_start(qa2[n][:], z0_d[:])

            # --- prologue: scratch memset, image-0 x, conv1 weights, then
            # PE warm-up matmuls that cover the x-quant latency.
            nc.gpsimd.memset(wscr[:], 0)
            qs1_t = [None] * NP_

            def xq_compute(n):
                # quantize x into a contiguous staging tile
                load_x(n)
                fill_planes(n)
                qs1 = qs1_t[n] = spool.tile([128, NCH, H, W], dt_act,
                                            name="qs1", tag="qs1")
                for j in range(NCH):
                    xt = x_sb[n][j]
                    if n < XQV:
                        m1 = mpool.tile([128, H, W], dt.bfloat16, name="m1",
                                        tag="xv1")
                        m2 = mpool.tile([128, H, W], dt.bfloat16, name="m2",
                                        tag="xv2")
                        nc.vector.tensor_scalar(
                            m1[:], xt[:], XQT[0], None, ALU.is_gt)
                        nc.vector.scalar_tensor_tensor(
                            m2[:], xt[:], XQT[1], m1[:], ALU.is_gt, ALU.add)
                        nc.vector.scalar_tensor_tensor(
                            qs1[:, j], xt[:], XQT[2], m2[:],
                            ALU.is_gt, ALU.add)
                    else:
                        s1 = mpool.tile([128, H, W], dt.bfloat16, name="s1",
                                        tag="xg1")
                        s2m = mpool.tile([128, H, W], dt.bfloat16, name="s2m",
                                         tag="xg2")
                        s3 = mpool.tile([128, H, W], dt.bfloat16, name="s3",
                                        tag="xg3")
                        st = mpool.tile([128, H, W], dt.bfloat16, name="st",
                                        tag="xg4")
                        for k, sk in enumerate((s1, s2m, s3)):
                            nc.scalar.activation(
                                sk[:], xt[:],
                                mybir.ActivationFunctionType.Sign,
                                bias=xqtn_sb[:, k:k + 1])
                        nc.gpsimd.tensor_tensor(st[:], s1[:], s2m[:], ALU.add)
                        nc.gpsimd.tensor_tensor(qs1[:, j], st[:], s3[:],
                                                ALU.add)

            def xq_scatter(n):
                for j in range(NCH):
                    nc.sync.dma_start(plane_interior(qa1[n], j),
                                      qs1_t[n][:, j])

            def conv_mms2(ps, w_sb, qa_n, co):
                # tap-major over both h-halves: consecutive matmuls share
                # one stationary lhsT so the PE's background weight buffer
                # hides the DoubleRow LDWEIGHTS behind the moving stream.
                for k in range(9):
                    dy, dx = divmod(k, 3)
                    for h in range(2):
                        off = (h * HALF + dy) * PW + dx
                        nc.tensor.matmul(
                            ps[:, h, 0:RUN],
                            w_sb[:, 0:NCH, k, co * 128:(co + 1) * 128],
                            qa_n[:, 0:NCH, off:off + RUN],
                            start=(k == 0), stop=(k == 8),
                            perf_mode=mybir.MatmulPerfMode.DoubleRow,
                        )

            def psum_pair(name):
                # one [128, 2, 512] fp32 tile = two aligned PSUM banks;
                # each h-half is a legal single-bank matmul target and the
                # epilogue reads both halves in one 3D FD=840 op (the 2
                # garbage columns per row are computed and then skipped by
                # the plane scatter).
                ps = pspool.tile([128, NCH, PSH], dt.float32, name=name,
                                 tag="ps")
                return ps, ps[:, :, 0:HALF * PW]

            # conv1 -> bn1 -> relu -> quant, folded into 3 per-channel
            # is_gt thresholds accumulated with fused STT ops on DVE
            def conv1_image(n):
                thr = t1s_sb if n >= XQV else t1_sb
                qs2 = spool.tile([128, NCH, NCH, HALF * PW], dt_act,
                                 name="qs2", tag="qs2")
                for co in range(NCH):
                    ps, psv = psum_pair("ps1")
                    conv_mms2(ps, w1_sb, qa1[n], co)
                    e1 = mpool.tile([128, NCH, HALF * PW], dt.bfloat16,
                                    name="e1", tag="e1")
                    e2 = mpool.tile([128, NCH, HALF * PW], dt.bfloat16,
                                    name="e2", tag="e2")
                    nc.vector.tensor_scalar(
                        e1[:], psv, thr[:, co, 0:1], None, ALU.is_gt)
                    nc.vector.scalar_tensor_tensor(
                        e2[:], psv, thr[:, co, 1:2], e1[:], ALU.is_gt,
                        ALU.add)
                    nc.vector.scalar_tensor_tensor(
                        qs2[:, co], psv, thr[:, co, 2:3], e2[:], ALU.is_gt,
                        ALU.add)
                    # interior rows across both h-halves are uniformly
                    # 30-strided, so one 3-dim scatter covers this co
                    nc.sync.dma_start(
                        plane_interior(qa2[n], co),
                        qs2[:, co].rearrange(
                            "p g (r c) -> p (g r) c", c=PW)[:, :, 0:W])

            # conv2 -> bn2 -> +residual -> relu -> out:
            #   v = ps*s2 + x (DVE STT), out = relu(v + bb2) (ACT bias)
            def conv2_image(n):
                for co in range(NCH):
                    ps, psv = psum_pair("ps2")
                    conv_mms2(ps, w2_sb, qa2[n], co)
                    v = upool.tile([128, H, W], dt.float32, name="v", tag="v")
                    o = opool.tile([128, H, W], dt.float32, name="o", tag="o")
                    for h in range(2):
                        psvh = psv[:, h].rearrange(
                            "p (r c) -> p r c", c=PW)[:, :, 0:W]
                        nc.vector.scalar_tensor_tensor(
                            v[:][:, h * HALF:(h + 1) * HALF, :], psvh,
                            s2_sb[:, co:co + 1],
                            x_sb[n][co][:, h * HALF:(h + 1) * HALF, :],
                            ALU.mult, ALU.add)
                    nc.scalar.activation(
                        o[:], v[:], mybir.ActivationFunctionType.Relu,
                        bias=b2_sb[:, co:co + 1])
                    nc.sync.dma_start(
                        y_d[n, co * 128:(co + 1) * 128, :, :], o[:])

            # software-pipelined emission, x-quant running two images
            # ahead: on each in-order queue, instructions are emitted in
            # the order their dependencies become ready, so nothing
            # head-of-line blocks. The qa1 scatter of image n+2 is emitted
            # after conv2(n-1)'s stores, matching readiness order on sync.
            xq_compute(0)
            nc.sync.dma_start(w1_sb[:], w1_d[:])
            for i in range(NWARM):
                wps = pspool.tile([128, NCH, PSH], dt.float32, name="wps",
                                  tag="ps")
                nc.tensor.matmul(
                    wps[:, 0, 0:RUN],
                    wscr[:, 0:NCH, 0:128],
                    wscr[:, 0:NCH, 0:RUN],
                    start=True, stop=True,
                    perf_mode=mybir.MatmulPerfMode.DoubleRow,
                )
            nc.sync.dma_start(w2_sb[:], w2_d[:])
            nc.sync.dma_start(t1_sb[:], t1_d[:])
            nc.sync.dma_start(t1s_sb[:], t1s_d[:])
            nc.sync.dma_start(xqtn_sb[:], xqtn_d[:])
            nc.sync.dma_start(s2_sb[:], s2_d[:])
            nc.sync.dma_start(b2_sb[:], b2_d[:])
            if NP_ > 1:
                xq_compute(1)
            xq_scatter(0)
            if NP_ > 1:
                xq_scatter(1)
            for n in range(NP_):
                conv1_image(n)
                if n + 2 < NP_:
                    xq_compute(n + 2)
                if n >= 1:
                    conv2_image(n - 1)
                if n + 2 < NP_:
                    xq_scatter(n + 2)
            conv2_image(NP_ - 1)

    nc.compile()
    return nc


_CACHED = None


def _get_program():
    global _CACHED
    if _CACHED is None:
        _CACHED = _build_program()
    return _CACHED


def kernel(x, w1, g1, b1, m1, v1, w2, g2, b2, m2, v2):
    _install_ntff_hook_shim()
    from concourse.bass_utils import run_bass_kernel_spmd

    x = np.asarray(x, np.float32)
    host = _host_arrays(w1, g1, b1, m1, v1, w2, g2, b2, m2, v2)

    xs = x.reshape(NCORES, NPER, C, H, W)
    in_maps = [{"x": np.ascontiguousarray(xs[c]), **host}
               for c in range(NCORES)]

    nc = _get_program()
    res = run_bass_kernel_spmd(
        nc, in_maps, core_ids=list(range(NCORES)),
        trace=bool(int(os.environ.get("KERNEL_TRACE", "0"))),
    )
    kernel.last_results = res
    y = np.concatenate([res.results[c]["y"][None] for c in range(NCORES)], 0)
    return np.ascontiguousarray(y.reshape(64, C, H, W).astype(np.float32))


# revision 41
# speedup vs baseline: 1.3515x; 1.2911x over previous
"""Trainium2 Bass kernel for a 2-bit-quantized (DoReFa) ResNet BasicBlock.

Full (unsharded) numpy inputs -> full numpy output.

Design:
  - batch (64) is sharded 8 images/core across 8 NeuronCores (pure data
    parallel, weights/BN params replicated).
  - 2-bit quantization makes every conv input an exact small integer:
    acts*3 in {0..3} (or sign-coded 2*qa-3), weights*3 in {-3,-1,1,3}.
    Those are exact in fp8e4 and the PE accumulates in fp32, so both
    convs are bit-exact integer matmuls at fp8 DoubleRow speed. All
    scale factors (the /9, BN scale/shift) fold into per-channel
    epilogue constants on host.
  - each 3x3 conv = 9 shifted matmuls accumulated in PSUM over padded
    30-wide activation planes; the moving operand is a flat 418-element
    run across 14 plane rows. Matmuls are emitted tap-major across the
    two h-halves of an output-channel group so consecutive matmuls share
    one stationary lhsT: the PE's background weight buffer then hides
    the 162ns DoubleRow LDWEIGHTS behind 2x177ns of streaming, which is
    the difference between a 212ns and a 177ns matmul cadence.
  - each (image, co) conv uses one 2-bank-aligned psum tile [128,2,512]
    (h-half per bank) so the whole epilogue runs as FD=784 ops, halving
    the per-instruction fixed cost.
  - epilogues write contiguous staging tiles; a single DMA per image
    scatters them into the padded planes (strided writes cost the
    engines nothing).
  - conv1 epilogue: BN+ReLU+requant folds into 3 per-channel psum-space
    thresholds; qa2 = chained is_gt ops (tensor_scalar + 2 fused
    scalar_tensor_tensor) on DVE. conv2 epilogue: v = ps*s2 + x (fused
    DVE op), out = relu(v + bb2) (ACT bias), one store per (image, co).
  - x-quant: image 0 on DVE (is_gt chain, prologue critical path);
    other images on ScalarE (3 ACT Signs -> S = 2*qa-3, pad -3) with
    the two adds on GpSimd, using sign-space conv1 thresholds t1s.
  - a scratch-fed run of dummy DoubleRow matmuls at kernel start warms
    the PE HAM clock gate and covers the x-load + quantize prologue so
    the real matmul stream starts warm and never throttles.
  - weight quantization (tanh / global max / round) + BN folding is done
    on host: it is O(weights) = 0.6 MB, vs 118 GFLOP of conv on device.
"""

import os
import sys
import numpy as np


def _install_ntff_hook_shim():
    """Provide antenv.axon_hooks if the image lacks it, so
    run_bass_kernel_spmd(trace=True) can capture NTFF profiles through
    libaxon_pjrt.so. No-op if the real module exists or the .so is absent."""
    try:
        import antenv.axon_hooks  # noqa: F401
        return
    except ImportError:
        pass
    import contextlib
    import ctypes
    import types

    so_path = "/opt/axon/libaxon_pjrt.so"
    _hook = None
    if os.path.exists(so_path):
        try:
            lib = ctypes.CDLL(so_path)
        except OSError:
            lib = None
        if lib is not None and hasattr(lib, "axon_start_nrt_profile"):
            lib.axon_start_nrt_profile.argtypes = [
                ctypes.POINTER(ctypes.c_int64), ctypes.c_size_t]
            lib.axon_start_nrt_profile.restype = ctypes.c_int64
            lib.axon_stop_nrt_profile.argtypes = [ctypes.c_char_p]
            lib.axon_stop_nrt_profile.restype = ctypes.c_int64

            @contextlib.contextmanager
            def _hook(output_dir, device_ids):
                import jax
                jax.devices()
                if device_ids:
                    ids = (ctypes.c_int64 * len(device_ids))(*device_ids)
                    rc = lib.axon_start_nrt_profile(ids, len(device_ids))
                else:
                    rc = lib.axon_start_nrt_profile(None, 0)
                if rc != 0:
                    raise RuntimeError(f"axon_start_nrt_profile rc={rc}")
                try:
                    yield
                finally:
                    n = lib.axon_stop_nrt_profile(str(output_dir).encode())
                    print(f"profile: {n} file(s) written to {output_dir}",
                          file=sys.stderr)

    mod = types.ModuleType("antenv.axon_hooks")
    mod.get_axon_ntff_profile_hook = lambda: _hook
    mod.set_axon_ntff_profile_hook = lambda h: None
    sys.modules["antenv.axon_hooks"] = mod


NCORES = 8
NPER = 8          # images per core
C = 256
NCH = 2           # channel chunks of 128
H = W = 28
PH = H + 2        # padded plane 30x30
PW = 30           # plane row stride
QSTR = 960        # allocated plane stride (16B-aligned, >= PH*PW)
HALF = 14         # rows per psum half-tile
RUN = (HALF - 1) * PW + W   # 418-element flat moving-run per matmul
PSH = 512         # psum half stride (one full bank of fp32)
BN_EPS = 1e-5
NWARM = int(os.environ.get("KERNEL_NWARM", "58"))   # PE warm-up matmuls
XQV = int(os.environ.get("KERNEL_XQV", "2"))        # imgs with x-quant on DVE


def _quant_weight3(w):
    """Replicate reference _quant_weight in f32, scaled by 3 -> {-3,-1,1,3}."""
    w = np.asarray(w, np.float32)
    t = np.tanh(w)
    m = np.max(np.abs(t))
    t2 = t / (np.float32(2.0) * m) + np.float32(0.5)
    k = np.round(t2 * np.float32(3.0))          # round-half-even == jnp.round
    return (2.0 * k - 3.0).astype(np.float32)


def _fold_bn(g, b, m, v):
    inv = np.asarray(g, np.float64) / np.sqrt(np.asarray(v, np.float64) + BN_EPS)
    beta = np.asarray(b, np.float64) - np.asarray(m, np.float64) * inv
    return inv, beta


def _w_tiles(qw3, dt):
    # [O, I, 3, 3] -> [p=128, ci=2, k=9, O=256] so lhsT slices are
    # [128, 2, 128] interleaved chunks (fp8 DoubleRow).
    return np.ascontiguousarray(
        np.transpose(qw3.reshape(C, NCH, 128, 9), (2, 1, 3, 0))
    ).astype(dt)


def _host_arrays(w1, g1, b1, m1, v1, w2, g2, b2, m2, v2):
    from concourse import mybir
    qw3_1 = _quant_weight3(w1)
    qw3_2 = _quant_weight3(w2)
    inv1, beta1 = _fold_bn(g1, b1, m1, v1)
    inv2, beta2 = _fold_bn(g2, b2, m2, v2)

    act_np = mybir.dt.np(mybir.dt.float8e4)
    w1t = _w_tiles(qw3_1, act_np)
    w2t = _w_tiles(qw3_2, act_np)

    # conv1 psum P1 = 9*conv_true (exact int); y = P1*inv1/9 + beta1.
    # quant level k iff y > tau_k = (2k-1)/6, i.e. P1 > (tau_k-beta1)*9/inv1
    # (inv1 > 0 given g1=1, v1 > 0), so qa2 = sum_k is_gt(P1, t1_k).
    assert np.all(inv1 > 0), "bn1 scale must be positive for threshold fold"
    taus = np.array([1.0, 3.0, 5.0]) / 6.0
    t1 = ((taus[None, :] - beta1[:, None]) * 9.0 / inv1[:, None])  # [C, 3]
    # sign-coded images store S1 = 2*qa1-3 (pad -3), so P1s = 2*P1 - 3*K1f
    # and the thresholds become 2*t1 - 3*K1f per output channel.
    k1f = qw3_1.reshape(C, -1).sum(axis=1)[:, None]                # [C, 1]
    t1s = 2.0 * t1 - 3.0 * k1f

    def fold_t(t):
        return np.ascontiguousarray(
            t.reshape(NCH, 128, 3).transpose(1, 0, 2)).astype(np.float32)

    # conv2 on qa2 in {0..3} with zero padding: P2 = 9*conv2_true, so
    # y2 = P2*(inv2/9) + beta2 and out = relu(y2 + x).
    s2 = np.ascontiguousarray(
        (inv2 / 9.0).reshape(NCH, 128).T).astype(np.float32)
    bb2 = np.ascontiguousarray(
        beta2.reshape(NCH, 128).T).astype(np.float32)
    z0 = np.zeros((128, NCH, QSTR), act_np)
    zm3 = np.full((128, NCH, QSTR), -3.0, act_np)
    xqtn = np.broadcast_to(
        -np.array([1.0, 3.0, 5.0], np.float32) / 6.0, (128, 3)).copy()
    return {"w1t": w1t, "w2t": w2t, "t1": fold_t(t1), "t1s": fold_t(t1s),
            "s2": s2, "bb2": bb2, "z0": z0, "zm3": zm3, "xqtn": xqtn}


def _build_program(nper=NPER):
    from concourse import bacc, tile, mybir
    dt = mybir.dt
    dt_act = dt.float8e4
    ALU = mybir.AluOpType

    nc = bacc.Bacc("TRN2", target_bir_lowering=False, debug=False,
                   num_devices=NCORES)
    NP_ = nper

    x_d = nc.dram_tensor("x", [NP_, C, H, W], dt.float32, kind="ExternalInput")
    w1_d = nc.dram_tensor("w1t", [128, NCH, 9, C], dt_act, kind="ExternalInput")
    w2_d = nc.dram_tensor("w2t", [128, NCH, 9, C], dt_act, kind="ExternalInput")
    t1_d = nc.dram_tensor("t1", [128, NCH, 3], dt.float32, kind="ExternalInput")
    t1s_d = nc.dram_tensor("t1s", [128, NCH, 3], dt.float32,
                           kind="ExternalInput")
    s2_d = nc.dram_tensor("s2", [128, NCH], dt.float32, kind="ExternalInput")
    b2_d = nc.dram_tensor("bb2", [128, NCH], dt.float32, kind="ExternalInput")
    z0_d = nc.dram_tensor("z0", [128, NCH, QSTR], dt_act, kind="ExternalInput")
    zm3_d = nc.dram_tensor("zm3", [128, NCH, QSTR], dt_act,
                           kind="ExternalInput")
    xqtn_d = nc.dram_tensor("xqtn", [128, 3], dt.float32,
                            kind="ExternalInput")
    y_d = nc.dram_tensor("y", [NP_, C, H, W], dt.float32, kind="ExternalOutput")

    XQT = [1.0 / 6.0, 3.0 / 6.0, 5.0 / 6.0]   # act-quant thresholds for x

    with tile.TileContext(nc) as tc:
        with (
            tc.tile_pool(name="wpool", bufs=1) as wpool,
            tc.tile_pool(name="xpool", bufs=2 * NP_) as xpool,
            tc.tile_pool(name="qpool", bufs=NP_) as qpool,
            tc.tile_pool(name="spool", bufs=3) as spool,
            tc.tile_pool(name="mpool", bufs=4) as mpool,
            tc.tile_pool(name="upool", bufs=4) as upool,
            tc.tile_pool(name="opool", bufs=4) as opool,
            tc.tile_pool(name="pspool", bufs=4, space="PSUM") as pspool,
        ):
            w1_sb = wpool.tile([128, NCH, 9, C], dt_act, name="w1sb")
            w2_sb = wpool.tile([128, NCH, 9, C], dt_act, name="w2sb")
            t1_sb = wpool.tile([128, NCH, 3], dt.float32, name="t1sb")
            t1s_sb = wpool.tile([128, NCH, 3], dt.float32, name="t1ssb")
            xqtn_sb = wpool.tile([128, 3], dt.float32, name="xqtnsb")
            s2_sb = wpool.tile([128, NCH], dt.float32, name="s2sb")
            b2_sb = wpool.tile([128, NCH], dt.float32, name="b2sb")
            wscr = wpool.tile([128, NCH, 512], dt_act, name="wscr")
            # zero/-3 padded quantized-activation planes (flat, per image)
            qa1 = [qpool.tile([128, NCH, QSTR], dt_act, name=f"qa1_{n}",
                              tag="qa1") for n in range(NP_)]
            qa2 = [qpool.tile([128, NCH, QSTR], dt_act, name=f"qa2_{n}",
                              tag="qa2") for n in range(NP_)]

            def plane_interior(qa_t, j):
                # [128, 28, 28] view of chunk j's (1..28, 1..28) interior
                return qa_t[:, j, 31:31 + H * PW].rearrange(
                    "p (r c) -> p r c", c=PW)[:, :, 0:W]

            x_sb = [[None] * NCH for _ in range(NP_)]

            def load_x(n):
                for j in range(NCH):
                    xt = xpool.tile([128, H, W], dt.float32,
                                    name=f"x_{n}_{j}", tag="x")
                    nc.sync.dma_start(xt[:],
                                      x_d[n, j * 128:(j + 1) * 128, :, :])
                    x_sb[n][j] = xt

            def fill_planes(n):
                # all DMAs ride the sync HW-DGE ring: the gpsimd/scalar
                # SWDGE rings have 10us-class issue-to-completion latency
                sign_coded = n >= XQV
                nc.sync.dma_start(qa1[n][:],
                                  zm3_d[:] if sign_coded else z0_d[:])
                nc.sync.dma_start(qa2[n][:], z0_d[:])

            # --- prologue: scratch memset, image-0 x, conv1 weights, then
            # PE warm-up matmuls that cover the x-quant latency.
            nc.gpsimd.memset(wscr[:], 0)

            def xq_compute(n):
                # quantize x straight into the plane interiors (strided
                # engine writes beat DMA scatter: 28-byte rows make the
                # descriptor generation cost 3-5us of queue time per DMA)
                load_x(n)
                fill_planes(n)
                for j in range(NCH):
                    xt = x_sb[n][j]
                    qa_in = plane_interior(qa1[n], j)
                    if n < XQV:
                        m1 = mpool.tile([128, H, W], dt.bfloat16, name="m1",
                                        tag="xv1")
                        m2 = mpool.tile([128, H, W], dt.bfloat16, name="m2",
                                        tag="xv2")
                        nc.vector.tensor_scalar(
                            m1[:], xt[:], XQT[0], None, ALU.is_gt)
                        nc.vector.scalar_tensor_tensor(
                            m2[:], xt[:], XQT[1], m1[:], ALU.is_gt, ALU.add)
                        nc.vector.scalar_tensor_tensor(
                            qa_in, xt[:], XQT[2], m2[:],
                            ALU.is_gt, ALU.add)
                    else:
                        s1 = mpool.tile([128, H, W], dt.bfloat16, name="s1",
                                        tag="xg1")
                        s2m = mpool.tile([128, H, W], dt.bfloat16, name="s2m",
                                         tag="xg2")
                        s3 = mpool.tile([128, H, W], dt.bfloat16, name="s3",
                                        tag="xg3")
                        st = mpool.tile([128, H, W], dt.bfloat16, name="st",
                                        tag="xg4")
                        for k, sk in enumerate((s1, s2m, s3)):
                            nc.scalar.activation(
                                sk[:], xt[:],
                                mybir.ActivationFunctionType.Sign,
                                bias=xqtn_sb[:, k:k + 1])
                        nc.gpsimd.tensor_tensor(st[:], s1[:], s2m[:], ALU.add)
                        nc.gpsimd.tensor_tensor(qa_in, st[:], s3[:], ALU.add)

            def conv_mms2(ps, w_sb, qa_n, co):
                # tap-major over both h-halves: consecutive matmuls share
                # one stationary lhsT so the PE's background weight buffer
                # hides the DoubleRow LDWEIGHTS behind the moving stream.
                for k in range(9):
                    dy, dx = divmod(k, 3)
                    for h in range(2):
                        off = (h * HALF + dy) * PW + dx
                        nc.tensor.matmul(
                            ps[:, h, 0:RUN],
                            w_sb[:, 0:NCH, k, co * 128:(co + 1) * 128],
                            qa_n[:, 0:NCH, off:off + RUN],
                            start=(k == 0), stop=(k == 8),
                            perf_mode=mybir.MatmulPerfMode.DoubleRow,
                        )

            def psum_pair(name):
                # one [128, 2, 512] fp32 tile = two aligned PSUM banks;
                # each h-half is a legal single-bank matmul target and the
                # epilogue reads both halves in one 3D FD=840 op (the 2
                # garbage columns per row are computed and then skipped by
                # the plane scatter).
                ps = pspool.tile([128, NCH, PSH], dt.float32, name=name,
                                 tag="ps")
                return ps, ps[:, :, 0:HALF * PW]

            # conv1 -> bn1 -> relu -> quant, folded into 3 per-channel
            # is_gt thresholds accumulated with fused STT ops on DVE
            def conv1_image(n):
                thr = t1s_sb if n >= XQV else t1_sb
                qs2 = spool.tile([128, NCH, NCH, HALF * PW], dt_act,
                                 name="qs2", tag="qs2")
                for co in range(NCH):
                    ps, psv = psum_pair("ps1")
                    conv_mms2(ps, w1_sb, qa1[n], co)
                    e1 = mpool.tile([128, NCH, HALF * PW], dt.bfloat16,
                                    name="e1", tag="e1")
                    e2 = mpool.tile([128, NCH, HALF * PW], dt.bfloat16,
                                    name="e2", tag="e2")
                    nc.vector.tensor_scalar(
                        e1[:], psv, thr[:, co, 0:1], None, ALU.is_gt)
                    nc.vector.scalar_tensor_tensor(
                        e2[:], psv, thr[:, co, 1:2], e1[:], ALU.is_gt,
                        ALU.add)
                    nc.vector.scalar_tensor_tensor(
                        qs2[:, co], psv, thr[:, co, 2:3], e2[:], ALU.is_gt,
                        ALU.add)
                    # interior rows across both h-halves are uniformly
                    # 30-strided, so one 3-dim Pool copy lands this co's
                    # staging tile in the padded plane (engine copy beats
                    # DMA scatter for 28-byte strided rows)
                    nc.gpsimd.tensor_copy(
                        plane_interior(qa2[n], co),
                        qs2[:, co].rearrange(
                            "p g (r c) -> p (g r) c", c=PW)[:, :, 0:W])

            # conv2 -> bn2 -> +residual -> relu -> out:
            #   v = ps*s2 + x (DVE STT), out = relu(v + bb2) (ACT bias)
            def conv2_image(n):
                for co in range(NCH):
                    ps, psv = psum_pair("ps2")
                    conv_mms2(ps, w2_sb, qa2[n], co)
                    v = upool.tile([128, H, W], dt.float32, name="v", tag="v")
                    o = opool.tile([128, H, W], dt.float32, name="o", tag="o")
                    for h in range(2):
                        psvh = psv[:, h].rearrange(
                            "p (r c) -> p r c", c=PW)[:, :, 0:W]
                        nc.vector.scalar_tensor_tensor(
                            v[:][:, h * HALF:(h + 1) * HALF, :], psvh,
                            s2_sb[:, co:co + 1],
                            x_sb[n][co][:, h * HALF:(h + 1) * HALF, :],
                            ALU.mult, ALU.add)
                    nc.scalar.activation(
                        o[:], v[:], mybir.ActivationFunctionType.Relu,
                        bias=b2_sb[:, co:co + 1])
                    nc.sync.dma_start(
                        y_d[n, co * 128:(co + 1) * 128, :, :], o[:])

            # software-pipelined emission, x-quant running two images
            # ahead: on each in-order queue, instructions are emitted in
            # the order their dependencies become ready, so nothing
            # head-of-line blocks. The qa1 scatter of image n+2 is emitted
            # after conv2(n-1)'s stores, matching readiness order on sync.
            xq_compute(0)
            nc.sync.dma_start(w1_sb[:], w1_d[:])
            for i in range(NWARM):
                wps = pspool.tile([128, NCH, PSH], dt.float32, name="wps",
                                  tag="ps")
                nc.tensor.matmul(
                    wps[:, 0, 0:RUN],
                    wscr[:, 0:NCH, 0:128],
                    wscr[:, 0:NCH, 0:RUN],
                    start=True, stop=True,
                    perf_mode=mybir.MatmulPerfMode.DoubleRow,
                )
            nc.sync.dma_start(w2_sb[:], w2_d[:])
            nc.sync.dma_start(t1_sb[:], t1_d[:])
            nc.sync.dma_start(t1s_sb[:], t1s_d[:])
            nc.sync.dma_start(xqtn_sb[:], xqtn_d[:])
            nc.sync.dma_start(s2_sb[:], s2_d[:])
            nc.sync.dma_start(b2_sb[:], b2_d[:])
            if NP_ > 1:
                xq_compute(1)
            for n in range(NP_):
                conv1_image(n)
                if n + 2 < NP_:
                    xq_compute(n + 2)
                if n >= 1:
                    conv2_image(n - 1)
            conv2_image(NP_ - 1)

    nc.compile()
    return nc


_CACHED = None


def _get_program():
    global _CACHED
    if _CACHED is None:
        _CACHED = _build_program()
    return _CACHED


def kernel(x, w1, g1, b1, m1, v1, w2, g2, b2, m2, v2):
    _install_ntff_hook_shim()
    from concourse.bass_utils import run_bass_kernel_spmd

    x = np.asarray(x, np.float32)
    host = _host_arrays(w1, g1, b1, m1, v1, w2, g2, b2, m2, v2)

    xs = x.reshape(NCORES, NPER, C, H, W)
    in_maps = [{"x": np.ascontiguousarray(xs[c]), **host}
               for c in range(NCORES)]

    nc = _get_program()
    res = run_bass_kernel_spmd(
        nc, in_maps, core_ids=list(range(NCORES)),
        trace=bool(int(os.environ.get("KERNEL_TRACE", "0"))),
    )
    kernel.last_results = res
    y = np.concatenate([res.results[c]["y"][None] for c in range(NCORES)], 0)
    return np.ascontiguousarray(y.reshape(64, C, H, W).astype(np.float32))


# revision 44
# speedup vs baseline: 1.6247x; 1.2022x over previous
"""Trainium2 Bass kernel for a 2-bit-quantized (DoReFa) ResNet BasicBlock.

Full (unsharded) numpy inputs -> full numpy output.

Design:
  - batch (64) is sharded 8 images/core across 8 NeuronCores (pure data
    parallel, weights/BN params replicated).
  - 2-bit quantization makes every conv input an exact small integer:
    acts*3 in {0..3} (or sign-coded 2*qa-3), weights*3 in {-3,-1,1,3}.
    Those are exact in fp8e4 and the PE accumulates in fp32, so both
    convs are bit-exact integer matmuls at fp8 DoubleRow speed. All
    scale factors (the /9, BN scale/shift) fold into per-channel
    epilogue constants on host.
  - each 3x3 conv = 9 shifted matmuls accumulated in PSUM over padded
    30-wide activation planes; the moving operand is a flat 418-element
    run across 14 plane rows. Matmuls are emitted tap-major across the
    two h-halves of an output-channel group so consecutive matmuls share
    one stationary lhsT: the PE's background weight buffer then hides
    the 162ns DoubleRow LDWEIGHTS behind 2x177ns of streaming, which is
    the difference between a 212ns and a 177ns matmul cadence.
  - each (image, co) conv uses one 2-bank-aligned psum tile [128,2,512]
    (h-half per bank) so the whole epilogue runs as FD=784 ops, halving
    the per-instruction fixed cost.
  - epilogues write contiguous staging tiles; a single DMA per image
    scatters them into the padded planes (strided writes cost the
    engines nothing).
  - conv1 epilogue: BN+ReLU+requant folds into 3 per-channel psum-space
    thresholds; qa2 = chained is_gt ops (tensor_scalar + 2 fused
    scalar_tensor_tensor) on DVE. conv2 epilogue: v = ps*s2 + x (fused
    DVE op), out = relu(v + bb2) (ACT bias), one store per (image, co).
  - x-quant: image 0 on DVE (is_gt chain, prologue critical path);
    other images on ScalarE (3 ACT Signs -> S = 2*qa-3, pad -3) with
    the two adds on GpSimd, using sign-space conv1 thresholds t1s.
  - a scratch-fed run of dummy DoubleRow matmuls at kernel start warms
    the PE HAM clock gate and covers the x-load + quantize prologue so
    the real matmul stream starts warm and never throttles.
  - weight quantization (tanh / global max / round) + BN folding is done
    on host: it is O(weights) = 0.6 MB, vs 118 GFLOP of conv on device.
"""

import os
import sys
import numpy as np


def _install_ntff_hook_shim():
    """Provide antenv.axon_hooks if the image lacks it, so
    run_bass_kernel_spmd(trace=True) can capture NTFF profiles through
    libaxon_pjrt.so. No-op if the real module exists or the .so is absent."""
    try:
        import antenv.axon_hooks  # noqa: F401
        return
    except ImportError:
        pass
    import contextlib
    import ctypes
    import types

    so_path = "/opt/axon/libaxon_pjrt.so"
    _hook = None
    if os.path.exists(so_path):
        try:
            lib = ctypes.CDLL(so_path)
        except OSError:
            lib = None
        if lib is not None and hasattr(lib, "axon_start_nrt_profile"):
            lib.axon_start_nrt_profile.argtypes = [
                ctypes.POINTER(ctypes.c_int64), ctypes.c_size_t]
            lib.axon_start_nrt_profile.restype = ctypes.c_int64
            lib.axon_stop_nrt_profile.argtypes = [ctypes.c_char_p]
            lib.axon_stop_nrt_profile.restype = ctypes.c_int64

            @contextlib.contextmanager
            def _hook(output_dir, device_ids):
                import jax
                jax.devices()
                if device_ids:
                    ids = (ctypes.c_int64 * len(device_ids))(*device_ids)
                    rc = lib.axon_start_nrt_profile(ids, len(device_ids))
                else:
                    rc = lib.axon_start_nrt_profile(None, 0)
                if rc != 0:
                    raise RuntimeError(f"axon_start_nrt_profile rc={rc}")
                try:
                    yield
                finally:
                    n = lib.axon_stop_nrt_profile(str(output_dir).encode())
                    print(f"profile: {n} file(s) written to {output_dir}",
                          file=sys.stderr)

    mod = types.ModuleType("antenv.axon_hooks")
    mod.get_axon_ntff_profile_hook = lambda: _hook
    mod.set_axon_ntff_profile_hook = lambda h: None
    sys.modules["antenv.axon_hooks"] = mod


NCORES = 8
NPER = 8          # images per core
C = 256
NCH = 2           # channel chunks of 128
H = W = 28
PH = H + 2        # padded plane 30x30
PW = 30           # plane row stride
QSTR = 960        # allocated plane stride (16B-aligned, >= PH*PW)
HALF = 14         # rows per psum half-tile
RUN = (HALF - 1) * PW + W   # 418-element flat moving-run per matmul
PSH = 512         # psum half stride (one full bank of fp32)
BN_EPS = 1e-5
NWARM = int(os.environ.get("KERNEL_NWARM", "40"))   # PE warm-up matmuls
XQV = int(os.environ.get("KERNEL_XQV", "1"))        # imgs with x-quant on DVE


def _quant_weight3(w):
    """Replicate reference _quant_weight in f32, scaled by 3 -> {-3,-1,1,3}."""
    w = np.asarray(w, np.float32)
    t = np.tanh(w)
    m = np.max(np.abs(t))
    t2 = t / (np.float32(2.0) * m) + np.float32(0.5)
    k = np.round(t2 * np.float32(3.0))          # round-half-even == jnp.round
    return (2.0 * k - 3.0).astype(np.float32)


def _fold_bn(g, b, m, v):
    inv = np.asarray(g, np.float64) / np.sqrt(np.asarray(v, np.float64) + BN_EPS)
    beta = np.asarray(b, np.float64) - np.asarray(m, np.float64) * inv
    return inv, beta


def _w_tiles(qw3, dt):
    # [O, I, 3, 3] -> [p=128, ci=2, k=9, O=256] so lhsT slices are
    # [128, 2, 128] interleaved chunks (fp8 DoubleRow).
    return np.ascontiguousarray(
        np.transpose(qw3.reshape(C, NCH, 128, 9), (2, 1, 3, 0))
    ).astype(dt)


def _host_arrays(w1, g1, b1, m1, v1, w2, g2, b2, m2, v2):
    from concourse import mybir
    qw3_1 = _quant_weight3(w1)
    qw3_2 = _quant_weight3(w2)
    inv1, beta1 = _fold_bn(g1, b1, m1, v1)
    inv2, beta2 = _fold_bn(g2, b2, m2, v2)

    act_np = mybir.dt.np(mybir.dt.float8e4)
    w1t = _w_tiles(qw3_1, act_np)
    w2t = _w_tiles(qw3_2, act_np)

    # conv1 psum P1 = 9*conv_true (exact int); y = P1*inv1/9 + beta1.
    # quant level k iff y > tau_k = (2k-1)/6, i.e. P1 > (tau_k-beta1)*9/inv1
    # (inv1 > 0 given g1=1, v1 > 0), so qa2 = sum_k is_gt(P1, t1_k).
    assert np.all(inv1 > 0), "bn1 scale must be positive for threshold fold"
    taus = np.array([1.0, 3.0, 5.0]) / 6.0
    t1 = ((taus[None, :] - beta1[:, None]) * 9.0 / inv1[:, None])  # [C, 3]
    # sign-coded images store S1 = 2*qa1-3 (pad -3), so P1s = 2*P1 - 3*K1f
    # and the thresholds become 2*t1 - 3*K1f per output channel.
    k1f = qw3_1.reshape(C, -1).sum(axis=1)[:, None]                # [C, 1]
    t1s = 2.0 * t1 - 3.0 * k1f

    def fold_t(t):
        return np.ascontiguousarray(
            t.reshape(NCH, 128, 3).transpose(1, 0, 2)).astype(np.float32)

    # conv2 on qa2 in {0..3} with zero padding: P2 = 9*conv2_true, so
    # y2 = P2*(inv2/9) + beta2 and out = relu(y2 + x).
    s2 = np.ascontiguousarray(
        (inv2 / 9.0).reshape(NCH, 128).T).astype(np.float32)
    bb2 = np.ascontiguousarray(
        beta2.reshape(NCH, 128).T).astype(np.float32)
    z0 = np.zeros((128, NCH, QSTR), act_np)
    zm3 = np.full((128, NCH, QSTR), -3.0, act_np)
    xqtn = np.broadcast_to(
        -np.array([1.0, 3.0, 5.0], np.float32) / 6.0, (128, 3)).copy()
    return {"w1t": w1t, "w2t": w2t, "t1": fold_t(t1), "t1s": fold_t(t1s),
            "s2": s2, "bb2": bb2, "z0": z0, "zm3": zm3, "xqtn": xqtn}


def _build_program(nper=NPER):
    from concourse import bacc, tile, mybir
    dt = mybir.dt
    dt_act = dt.float8e4
    ALU = mybir.AluOpType

    nc = bacc.Bacc("TRN2", target_bir_lowering=False, debug=False,
                   num_devices=NCORES)
    NP_ = nper

    x_d = nc.dram_tensor("x", [NP_, C, H, W], dt.float32, kind="ExternalInput")
    w1_d = nc.dram_tensor("w1t", [128, NCH, 9, C], dt_act, kind="ExternalInput")
    w2_d = nc.dram_tensor("w2t", [128, NCH, 9, C], dt_act, kind="ExternalInput")
    t1_d = nc.dram_tensor("t1", [128, NCH, 3], dt.float32, kind="ExternalInput")
    t1s_d = nc.dram_tensor("t1s", [128, NCH, 3], dt.float32,
                           kind="ExternalInput")
    s2_d = nc.dram_tensor("s2", [128, NCH], dt.float32, kind="ExternalInput")
    b2_d = nc.dram_tensor("bb2", [128, NCH], dt.float32, kind="ExternalInput")
    z0_d = nc.dram_tensor("z0", [128, NCH, QSTR], dt_act, kind="ExternalInput")
    zm3_d = nc.dram_tensor("zm3", [128, NCH, QSTR], dt_act,
                           kind="ExternalInput")
    xqtn_d = nc.dram_tensor("xqtn", [128, 3], dt.float32,
                            kind="ExternalInput")
    y_d = nc.dram_tensor("y", [NP_, C, H, W], dt.float32, kind="ExternalOutput")

    XQT = [1.0 / 6.0, 3.0 / 6.0, 5.0 / 6.0]   # act-quant thresholds for x

    with tile.TileContext(nc) as tc:
        with (
            tc.tile_pool(name="wpool", bufs=1) as wpool,
            tc.tile_pool(name="xpool", bufs=2 * NP_) as xpool,
            tc.tile_pool(name="qpool", bufs=NP_) as qpool,
            tc.tile_pool(name="spool", bufs=3) as spool,
            tc.tile_pool(name="mpool", bufs=4) as mpool,
            tc.tile_pool(name="upool", bufs=4) as upool,
            tc.tile_pool(name="opool", bufs=4) as opool,
            tc.tile_pool(name="pspool", bufs=4, space="PSUM") as pspool,
        ):
            w1_sb = wpool.tile([128, NCH, 9, C], dt_act, name="w1sb")
            w2_sb = wpool.tile([128, NCH, 9, C], dt_act, name="w2sb")
            t1_sb = wpool.tile([128, NCH, 3], dt.float32, name="t1sb")
            t1s_sb = wpool.tile([128, NCH, 3], dt.float32, name="t1ssb")
            xqtn_sb = wpool.tile([128, 3], dt.float32, name="xqtnsb")
            s2_sb = wpool.tile([128, NCH], dt.float32, name="s2sb")
            b2_sb = wpool.tile([128, NCH], dt.float32, name="b2sb")
            wscr = wpool.tile([128, NCH, 512], dt_act, name="wscr")
            # zero/-3 padded quantized-activation planes (flat, per image)
            qa1 = [qpool.tile([128, NCH, QSTR], dt_act, name=f"qa1_{n}",
                              tag="qa1") for n in range(NP_)]
            qa2 = [qpool.tile([128, NCH, QSTR], dt_act, name=f"qa2_{n}",
                              tag="qa2") for n in range(NP_)]

            def plane_interior(qa_t, j):
                # [128, 28, 28] view of chunk j's (1..28, 1..28) interior
                return qa_t[:, j, 31:31 + H * PW].rearrange(
                    "p (r c) -> p r c", c=PW)[:, :, 0:W]

            x_sb = [[None] * NCH for _ in range(NP_)]

            def load_x(n):
                for j in range(NCH):
                    xt = xpool.tile([128, H, W], dt.float32,
                                    name=f"x_{n}_{j}", tag="x")
                    nc.sync.dma_start(xt[:],
                                      x_d[n, j * 128:(j + 1) * 128, :, :])
                    x_sb[n][j] = xt

            def fill_planes(n):
                # all DMAs ride the sync HW-DGE ring: the gpsimd/scalar
                # SWDGE rings have 10us-class issue-to-completion latency
                sign_coded = n >= XQV
                nc.sync.dma_start(qa1[n][:],
                                  zm3_d[:] if sign_coded else z0_d[:])
                nc.sync.dma_start(qa2[n][:], z0_d[:])

            # --- prologue: scratch memset, image-0 x, conv1 weights, then
            # PE warm-up matmuls that cover the x-quant latency.
            nc.gpsimd.memset(wscr[:], 0)

            def xq_compute(n):
                # quantize x straight into the plane interiors (strided
                # engine writes beat DMA scatter: 28-byte rows make the
                # descriptor generation cost 3-5us of queue time per DMA)
                if n >= 2:
                    load_x(n)
                    fill_planes(n)
                for j in range(NCH):
                    xt = x_sb[n][j]
                    qa_in = plane_interior(qa1[n], j)
                    if n < XQV:
                        m1 = mpool.tile([128, H, W], dt.bfloat16, name="m1",
                                        tag="xv1")
                        m2 = mpool.tile([128, H, W], dt.bfloat16, name="m2",
                                        tag="xv2")
                        nc.vector.tensor_scalar(
                            m1[:], xt[:], XQT[0], None, ALU.is_gt)
                        nc.vector.scalar_tensor_tensor(
                            m2[:], xt[:], XQT[1], m1[:], ALU.is_gt, ALU.add)
                        nc.vector.scalar_tensor_tensor(
                            qa_in, xt[:], XQT[2], m2[:],
                            ALU.is_gt, ALU.add)
                    else:
                        s1 = mpool.tile([128, H, W], dt.bfloat16, name="s1",
                                        tag="xg1")
                        s2m = mpool.tile([128, H, W], dt.bfloat16, name="s2m",
                                         tag="xg2")
                        s3 = mpool.tile([128, H, W], dt.bfloat16, name="s3",
                                        tag="xg3")
                        st = mpool.tile([128, H, W], dt.bfloat16, name="st",
                                        tag="xg4")
                        for k, sk in enumerate((s1, s2m, s3)):
                            nc.scalar.activation(
                                sk[:], xt[:],
                                mybir.ActivationFunctionType.Sign,
                                bias=xqtn_sb[:, k:k + 1])
                        nc.gpsimd.tensor_tensor(st[:], s1[:], s2m[:], ALU.add)
                        nc.gpsimd.tensor_tensor(qa_in, st[:], s3[:], ALU.add)

            def conv_mms2(ps, w_sb, qa_n, co):
                # tap-major over both h-halves: consecutive matmuls share
                # one stationary lhsT so the PE's background weight buffer
                # hides the DoubleRow LDWEIGHTS behind the moving stream.
                for k in range(9):
                    dy, dx = divmod(k, 3)
                    for h in range(2):
                        off = (h * HALF + dy) * PW + dx
                        nc.tensor.matmul(
                            ps[:, h, 0:RUN],
                            w_sb[:, 0:NCH, k, co * 128:(co + 1) * 128],
                            qa_n[:, 0:NCH, off:off + RUN],
                            start=(k == 0), stop=(k == 8),
                            perf_mode=mybir.MatmulPerfMode.DoubleRow,
                        )

            def psum_pair(name):
                # one [128, 2, 512] fp32 tile = two aligned PSUM banks;
                # each h-half is a legal single-bank matmul target and the
                # epilogue reads both halves in one 3D FD=840 op (the 2
                # garbage columns per row are computed and then skipped by
                # the plane scatter).
                ps = pspool.tile([128, NCH, PSH], dt.float32, name=name,
                                 tag="ps")
                return ps, ps[:, :, 0:HALF * PW]

            # conv1 -> bn1 -> relu -> quant, folded into 3 per-channel
            # is_gt thresholds accumulated with fused STT ops on DVE
            def conv1_image(n):
                thr = t1s_sb if n >= XQV else t1_sb
                qs2 = spool.tile([128, NCH, NCH, HALF * PW], dt_act,
                                 name="qs2", tag="qs2")
                for co in range(NCH):
                    ps, psv = psum_pair("ps1")
                    conv_mms2(ps, w1_sb, qa1[n], co)
                    e1 = mpool.tile([128, NCH, HALF * PW], dt.bfloat16,
                                    name="e1", tag="e1")
                    e2 = mpool.tile([128, NCH, HALF * PW], dt.bfloat16,
                                    name="e2", tag="e2")
                    nc.vector.tensor_scalar(
                        e1[:], psv, thr[:, co, 0:1], None, ALU.is_gt)
                    nc.vector.scalar_tensor_tensor(
                        e2[:], psv, thr[:, co, 1:2], e1[:], ALU.is_gt,
                        ALU.add)
                    nc.vector.scalar_tensor_tensor(
                        qs2[:, co], psv, thr[:, co, 2:3], e2[:], ALU.is_gt,
                        ALU.add)
                    # interior rows across both h-halves are uniformly
                    # 30-strided, so one 3-dim Pool copy lands this co's
                    # staging tile in the padded plane (engine copy beats
                    # DMA scatter for 28-byte strided rows)
                    nc.gpsimd.tensor_copy(
                        plane_interior(qa2[n], co),
                        qs2[:, co].rearrange(
                            "p g (r c) -> p (g r) c", c=PW)[:, :, 0:W])

            # conv2 -> bn2 -> +residual -> relu -> out:
            #   v = ps*s2 + x (DVE STT), out = relu(v + bb2) (ACT bias)
            def conv2_image(n):
                for co in range(NCH):
                    ps, psv = psum_pair("ps2")
                    conv_mms2(ps, w2_sb, qa2[n], co)
                    v = upool.tile([128, H, W], dt.float32, name="v", tag="v")
                    o = opool.tile([128, H, W], dt.float32, name="o", tag="o")
                    for h in range(2):
                        psvh = psv[:, h].rearrange(
                            "p (r c) -> p r c", c=PW)[:, :, 0:W]
                        nc.vector.scalar_tensor_tensor(
                            v[:][:, h * HALF:(h + 1) * HALF, :], psvh,
                            s2_sb[:, co:co + 1],
                            x_sb[n][co][:, h * HALF:(h + 1) * HALF, :],
                            ALU.mult, ALU.add)
                    nc.scalar.activation(
                        o[:], v[:], mybir.ActivationFunctionType.Relu,
                        bias=b2_sb[:, co:co + 1])
                    nc.sync.dma_start(
                        y_d[n, co * 128:(co + 1) * 128, :, :], o[:])

            # software-pipelined emission, x-quant running two images
            # ahead: on each in-order queue, instructions are emitted in
            # the order their dependencies become ready, so nothing
            # head-of-line blocks. The qa1 scatter of image n+2 is emitted
            # after conv2(n-1)'s stores, matching readiness order on sync.
            # image 0+1 x loads lead the sync ring; weights/params follow
            load_x(0)
            if NP_ > 1:
                load_x(1)
            fill_planes(0)
            if NP_ > 1:
                fill_planes(1)
            nc.sync.dma_start(w1_sb[:], w1_d[:])
            for i in range(NWARM):
                wps = pspool.tile([128, NCH, PSH], dt.float32, name="wps",
                                  tag="ps")
                nc.tensor.matmul(
                    wps[:, 0, 0:RUN],
                    wscr[:, 0:NCH, 0:128],
                    wscr[:, 0:NCH, 0:RUN],
                    start=True, stop=True,
                    perf_mode=mybir.MatmulPerfMode.DoubleRow,
                )
            xq_compute(0)
            nc.sync.dma_start(w2_sb[:], w2_d[:])
            nc.sync.dma_start(t1_sb[:], t1_d[:])
            nc.sync.dma_start(t1s_sb[:], t1s_d[:])
            nc.sync.dma_start(xqtn_sb[:], xqtn_d[:])
            nc.sync.dma_start(s2_sb[:], s2_d[:])
            nc.sync.dma_start(b2_sb[:], b2_d[:])
            if NP_ > 1:
                xq_compute(1)
            for n in range(NP_):
                conv1_image(n)
                if n + 2 < NP_:
                    xq_compute(n + 2)
                if n >= 1:
                    conv2_image(n - 1)
            conv2_image(NP_ - 1)

    nc.compile()
    return nc


_CACHED = None


def _get_program():
    global _CACHED
    if _CACHED is None:
        _CACHED = _build_program()
    return _CACHED


def kernel(x, w1, g1, b1, m1, v1, w2, g2, b2, m2, v2):
    _install_ntff_hook_shim()
    from concourse.bass_utils import run_bass_kernel_spmd

    x = np.asarray(x, np.float32)
    host = _host_arrays(w1, g1, b1, m1, v1, w2, g2, b2, m2, v2)

    xs = x.reshape(NCORES, NPER, C, H, W)
    in_maps = [{"x": np.ascontiguousarray(xs[c]), **host}
               for c in range(NCORES)]

    nc = _get_program()
    res = run_bass_kernel_spmd(
        nc, in_maps, core_ids=list(range(NCORES)),
        trace=bool(int(os.environ.get("KERNEL_TRACE", "0"))),
    )
    kernel.last_results = res
    y = np.concatenate([res.results[c]["y"][None] for c in range(NCORES)], 0)
    return np.ascontiguousarray(y.reshape(64, C, H, W).astype(np.float32))


# revision 46
# speedup vs baseline: 1.8659x; 1.1485x over previous
"""Trainium2 Bass kernel for a 2-bit-quantized (DoReFa) ResNet BasicBlock.

Full (unsharded) numpy inputs -> full numpy output.

Design:
  - batch (64) is sharded 8 images/core across 8 NeuronCores (pure data
    parallel, weights/BN params replicated).
  - 2-bit quantization makes every conv input an exact small integer:
    acts*3 in {0..3} (or sign-coded 2*qa-3), weights*3 in {-3,-1,1,3}.
    Those are exact in fp8e4 and the PE accumulates in fp32, so both
    convs are bit-exact integer matmuls at fp8 DoubleRow speed. All
    scale factors (the /9, BN scale/shift) fold into per-channel
    epilogue constants on host.
  - each 3x3 conv = 9 shifted matmuls accumulated in PSUM over padded
    30-wide activation planes; the moving operand is a flat 418-element
    run across 14 plane rows. Matmuls are emitted tap-major across the
    two h-halves of an output-channel group so consecutive matmuls share
    one stationary lhsT: the PE's background weight buffer then hides
    the 162ns DoubleRow LDWEIGHTS behind 2x177ns of streaming, which is
    the difference between a 212ns and a 177ns matmul cadence.
  - each (image, co) conv uses one 2-bank-aligned psum tile [128,2,512]
    (h-half per bank) so the whole epilogue runs as FD=784 ops, halving
    the per-instruction fixed cost.
  - epilogues write contiguous staging tiles; a single DMA per image
    scatters them into the padded planes (strided writes cost the
    engines nothing).
  - conv1 epilogue: BN+ReLU+requant folds into 3 per-channel psum-space
    thresholds; qa2 = chained is_gt ops (tensor_scalar + 2 fused
    scalar_tensor_tensor) on DVE. conv2 epilogue: v = ps*s2 + x (fused
    DVE op), out = relu(v + bb2) (ACT bias), one store per (image, co).
  - x-quant: image 0 on DVE (is_gt chain, prologue critical path);
    other images on ScalarE (3 ACT Signs -> S = 2*qa-3, pad -3) with
    the two adds on GpSimd, using sign-space conv1 thresholds t1s.
  - a scratch-fed run of dummy DoubleRow matmuls at kernel start warms
    the PE HAM clock gate and covers the x-load + quantize prologue so
    the real matmul stream starts warm and never throttles.
  - weight quantization (tanh / global max / round) + BN folding is done
    on host: it is O(weights) = 0.6 MB, vs 118 GFLOP of conv on device.
"""

import os
import sys
import numpy as np


def _install_ntff_hook_shim():
    """Provide antenv.axon_hooks if the image lacks it, so
    run_bass_kernel_spmd(trace=True) can capture NTFF profiles through
    libaxon_pjrt.so. No-op if the real module exists or the .so is absent."""
    try:
        import antenv.axon_hooks  # noqa: F401
        return
    except ImportError:
        pass
    import contextlib
    import ctypes
    import types

    so_path = "/opt/axon/libaxon_pjrt.so"
    _hook = None
    if os.path.exists(so_path):
        try:
            lib = ctypes.CDLL(so_path)
        except OSError:
            lib = None
        if lib is not None and hasattr(lib, "axon_start_nrt_profile"):
            lib.axon_start_nrt_profile.argtypes = [
                ctypes.POINTER(ctypes.c_int64), ctypes.c_size_t]
            lib.axon_start_nrt_profile.restype = ctypes.c_int64
            lib.axon_stop_nrt_profile.argtypes = [ctypes.c_char_p]
            lib.axon_stop_nrt_profile.restype = ctypes.c_int64

            @contextlib.contextmanager
            def _hook(output_dir, device_ids):
                import jax
                jax.devices()
                if device_ids:
                    ids = (ctypes.c_int64 * len(device_ids))(*device_ids)
                    rc = lib.axon_start_nrt_profile(ids, len(device_ids))
                else:
                    rc = lib.axon_start_nrt_profile(None, 0)
                if rc != 0:
                    raise RuntimeError(f"axon_start_nrt_profile rc={rc}")
                try:
                    yield
                finally:
                    n = lib.axon_stop_nrt_profile(str(output_dir).encode())
                    print(f"profile: {n} file(s) written to {output_dir}",
                          file=sys.stderr)

    mod = types.ModuleType("antenv.axon_hooks")
    mod.get_axon_ntff_profile_hook = lambda: _hook
    mod.set_axon_ntff_profile_hook = lambda h: None
    sys.modules["antenv.axon_hooks"] = mod


NCORES = 8
NPER = 8          # images per core
C = 256
NCH = 2           # channel chunks of 128
H = W = 28
PH = H + 2        # padded plane 30x30
PW = 30           # plane row stride
QSTR = 960        # allocated plane stride (16B-aligned, >= PH*PW)
HALF = 14         # rows per psum half-tile
RUN = (HALF - 1) * PW + W   # 418-element flat moving-run per matmul
PSH = 512         # psum half stride (one full bank of fp32)
BN_EPS = 1e-5
NWARM = int(os.environ.get("KERNEL_NWARM", "40"))   # PE warm-up matmuls
XQV = int(os.environ.get("KERNEL_XQV", "1"))        # imgs with x-quant on DVE


def _quant_weight3(w):
    """Replicate reference _quant_weight in f32, scaled by 3 -> {-3,-1,1,3}."""
    w = np.asarray(w, np.float32)
    t = np.tanh(w)
    m = np.max(np.abs(t))
    t2 = t / (np.float32(2.0) * m) + np.float32(0.5)
    k = np.round(t2 * np.float32(3.0))          # round-half-even == jnp.round
    return (2.0 * k - 3.0).astype(np.float32)


def _fold_bn(g, b, m, v):
    inv = np.asarray(g, np.float64) / np.sqrt(np.asarray(v, np.float64) + BN_EPS)
    beta = np.asarray(b, np.float64) - np.asarray(m, np.float64) * inv
    return inv, beta


def _w_tiles(qw3, dt):
    # [O, I, 3, 3] -> [p=128, ci=2, k=9, O=256] so lhsT slices are
    # [128, 2, 128] interleaved chunks (fp8 DoubleRow).
    return np.ascontiguousarray(
        np.transpose(qw3.reshape(C, NCH, 128, 9), (2, 1, 3, 0))
    ).astype(dt)


def _host_arrays(w1, g1, b1, m1, v1, w2, g2, b2, m2, v2):
    from concourse import mybir
    qw3_1 = _quant_weight3(w1)
    qw3_2 = _quant_weight3(w2)
    inv1, beta1 = _fold_bn(g1, b1, m1, v1)
    inv2, beta2 = _fold_bn(g2, b2, m2, v2)

    act_np = mybir.dt.np(mybir.dt.float8e4)
    w1t = _w_tiles(qw3_1, act_np)
    w2t = _w_tiles(qw3_2, act_np)

    # conv1 psum P1 = 9*conv_true (exact int); y = P1*inv1/9 + beta1.
    # quant level k iff y > tau_k = (2k-1)/6, i.e. P1 > (tau_k-beta1)*9/inv1
    # (inv1 > 0 given g1=1, v1 > 0), so qa2 = sum_k is_gt(P1, t1_k).
    assert np.all(inv1 > 0), "bn1 scale must be positive for threshold fold"
    taus = np.array([1.0, 3.0, 5.0]) / 6.0
    t1 = ((taus[None, :] - beta1[:, None]) * 9.0 / inv1[:, None])  # [C, 3]
    # sign-coded images store S1 = 2*qa1-3 (pad -3), so P1s = 2*P1 - 3*K1f
    # and the thresholds become 2*t1 - 3*K1f per output channel.
    k1f = qw3_1.reshape(C, -1).sum(axis=1)[:, None]                # [C, 1]
    t1s = 2.0 * t1 - 3.0 * k1f

    def fold_t(t):
        return np.ascontiguousarray(
            t.reshape(NCH, 128, 3).transpose(1, 0, 2)).astype(np.float32)

    # conv2 on qa2 in {0..3} with zero padding: P2 = 9*conv2_true, so
    # y2 = P2*(inv2/9) + beta2 and out = relu(y2 + x).
    s2 = np.ascontiguousarray(
        (inv2 / 9.0).reshape(NCH, 128).T).astype(np.float32)
    bb2 = np.ascontiguousarray(
        beta2.reshape(NCH, 128).T).astype(np.float32)
    z0 = np.zeros((128, NCH, QSTR), act_np)
    zm3 = np.full((128, NCH, QSTR), -3.0, act_np)
    xqtn = np.broadcast_to(
        -np.array([1.0, 3.0, 5.0], np.float32) / 6.0, (128, 3)).copy()
    return {"w1t": w1t, "w2t": w2t, "t1": fold_t(t1), "t1s": fold_t(t1s),
            "s2": s2, "bb2": bb2, "z0": z0, "zm3": zm3, "xqtn": xqtn}


def _build_program(nper=NPER):
    from concourse import bacc, tile, mybir
    dt = mybir.dt
    dt_act = dt.float8e4
    ALU = mybir.AluOpType

    nc = bacc.Bacc("TRN2", target_bir_lowering=False, debug=False,
                   num_devices=NCORES)
    NP_ = nper

    x_d = nc.dram_tensor("x", [NP_, C, H, W], dt.float32, kind="ExternalInput")
    w1_d = nc.dram_tensor("w1t", [128, NCH, 9, C], dt_act, kind="ExternalInput")
    w2_d = nc.dram_tensor("w2t", [128, NCH, 9, C], dt_act, kind="ExternalInput")
    t1_d = nc.dram_tensor("t1", [128, NCH, 3], dt.float32, kind="ExternalInput")
    t1s_d = nc.dram_tensor("t1s", [128, NCH, 3], dt.float32,
                           kind="ExternalInput")
    s2_d = nc.dram_tensor("s2", [128, NCH], dt.float32, kind="ExternalInput")
    b2_d = nc.dram_tensor("bb2", [128, NCH], dt.float32, kind="ExternalInput")
    z0_d = nc.dram_tensor("z0", [128, NCH, QSTR], dt_act, kind="ExternalInput")
    zm3_d = nc.dram_tensor("zm3", [128, NCH, QSTR], dt_act,
                           kind="ExternalInput")
    xqtn_d = nc.dram_tensor("xqtn", [128, 3], dt.float32,
                            kind="ExternalInput")
    y_d = nc.dram_tensor("y", [NP_, C, H, W], dt.float32, kind="ExternalOutput")

    XQT = [1.0 / 6.0, 3.0 / 6.0, 5.0 / 6.0]   # act-quant thresholds for x

    with tile.TileContext(nc) as tc:
        with (
            tc.tile_pool(name="wpool", bufs=1) as wpool,
            tc.tile_pool(name="xpool", bufs=2 * NP_) as xpool,
            tc.tile_pool(name="qpool", bufs=NP_) as qpool,
            tc.tile_pool(name="spool", bufs=3) as spool,
            tc.tile_pool(name="mpool", bufs=4) as mpool,
            tc.tile_pool(name="upool", bufs=4) as upool,
            tc.tile_pool(name="opool", bufs=4) as opool,
            tc.tile_pool(name="pspool", bufs=4, space="PSUM") as pspool,
        ):
            w1_sb = wpool.tile([128, NCH, 9, C], dt_act, name="w1sb")
            w2_sb = wpool.tile([128, NCH, 9, C], dt_act, name="w2sb")
            t1_sb = wpool.tile([128, NCH, 3], dt.float32, name="t1sb")
            t1s_sb = wpool.tile([128, NCH, 3], dt.float32, name="t1ssb")
            xqtn_sb = wpool.tile([128, 3], dt.float32, name="xqtnsb")
            s2_sb = wpool.tile([128, NCH], dt.float32, name="s2sb")
            b2_sb = wpool.tile([128, NCH], dt.float32, name="b2sb")
            wscr = wpool.tile([128, NCH, 512], dt_act, name="wscr")
            # zero/-3 padded quantized-activation planes (flat, per image)
            qa1 = [qpool.tile([128, NCH, QSTR], dt_act, name=f"qa1_{n}",
                              tag="qa1") for n in range(NP_)]
            qa2 = [qpool.tile([128, NCH, QSTR], dt_act, name=f"qa2_{n}",
                              tag="qa2") for n in range(NP_)]

            def plane_interior(qa_t, j):
                # [128, 28, 28] view of chunk j's (1..28, 1..28) interior
                return qa_t[:, j, 31:31 + H * PW].rearrange(
                    "p (r c) -> p r c", c=PW)[:, :, 0:W]

            x_sb = [[None] * NCH for _ in range(NP_)]

            def load_x(n):
                for j in range(NCH):
                    xt = xpool.tile([128, H, W], dt.float32,
                                    name=f"x_{n}_{j}", tag="x")
                    nc.sync.dma_start(xt[:],
                                      x_d[n, j * 128:(j + 1) * 128, :, :])
                    x_sb[n][j] = xt

            def fill_planes(n):
                # all DMAs ride the sync HW-DGE ring: the gpsimd/scalar
                # SWDGE rings have 10us-class issue-to-completion latency
                sign_coded = n >= XQV
                nc.sync.dma_start(qa1[n][:],
                                  zm3_d[:] if sign_coded else z0_d[:])
                nc.sync.dma_start(qa2[n][:], z0_d[:])

            # --- prologue: scratch memset, image-0 x, conv1 weights, then
            # PE warm-up matmuls that cover the x-quant latency.
            nc.gpsimd.memset(wscr[:], 0)

            def xq_compute(n):
                # quantize x straight into the plane interiors (strided
                # engine writes beat DMA scatter: 28-byte rows make the
                # descriptor generation cost 3-5us of queue time per DMA)
                if n >= 2:
                    load_x(n)
                    fill_planes(n)
                for j in range(NCH):
                    xt = x_sb[n][j]
                    qa_in = plane_interior(qa1[n], j)
                    if n < XQV:
                        m1 = mpool.tile([128, H, W], dt.bfloat16, name="m1",
                                        tag="xv1")
                        m2 = mpool.tile([128, H, W], dt.bfloat16, name="m2",
                                        tag="xv2")
                        nc.vector.tensor_scalar(
                            m1[:], xt[:], XQT[0], None, ALU.is_gt)
                        nc.vector.scalar_tensor_tensor(
                            m2[:], xt[:], XQT[1], m1[:], ALU.is_gt, ALU.add)
                        nc.vector.scalar_tensor_tensor(
                            qa_in, xt[:], XQT[2], m2[:],
                            ALU.is_gt, ALU.add)
                    else:
                        s1 = mpool.tile([128, H, W], dt.bfloat16, name="s1",
                                        tag="xg1")
                        s2m = mpool.tile([128, H, W], dt.bfloat16, name="s2m",
                                         tag="xg2")
                        s3 = mpool.tile([128, H, W], dt.bfloat16, name="s3",
                                        tag="xg3")
                        st = mpool.tile([128, H, W], dt.bfloat16, name="st",
                                        tag="xg4")
                        for k, sk in enumerate((s1, s2m, s3)):
                            nc.scalar.activation(
                                sk[:], xt[:],
                                mybir.ActivationFunctionType.Sign,
                                bias=xqtn_sb[:, k:k + 1])
                        nc.gpsimd.tensor_tensor(st[:], s1[:], s2m[:], ALU.add)
                        nc.gpsimd.tensor_tensor(qa_in, st[:], s3[:], ALU.add)

            def conv_mms2(ps, w_sb, qa_n, co):
                # tap-major over both h-halves: consecutive matmuls share
                # one stationary lhsT so the PE's background weight buffer
                # hides the DoubleRow LDWEIGHTS behind the moving stream.
                for k in range(9):
                    dy, dx = divmod(k, 3)
                    for h in range(2):
                        off = (h * HALF + dy) * PW + dx
                        nc.tensor.matmul(
                            ps[:, h, 0:RUN],
                            w_sb[:, 0:NCH, k, co * 128:(co + 1) * 128],
                            qa_n[:, 0:NCH, off:off + RUN],
                            start=(k == 0), stop=(k == 8),
                            perf_mode=mybir.MatmulPerfMode.DoubleRow,
                        )

            def psum_pair(name):
                # one [128, 2, 512] fp32 tile = two aligned PSUM banks;
                # each h-half is a legal single-bank matmul target and the
                # epilogue reads both halves in one 3D FD=840 op (the 2
                # garbage columns per row are computed and then skipped by
                # the plane scatter).
                ps = pspool.tile([128, NCH, PSH], dt.float32, name=name,
                                 tag="ps")
                return ps, ps[:, :, 0:HALF * PW]

            # conv1 -> bn1 -> relu -> quant, folded into 3 per-channel
            # is_gt thresholds accumulated with fused STT ops on DVE
            def conv1_image(n):
                thr = t1s_sb if n >= XQV else t1_sb
                qs2 = spool.tile([128, NCH, NCH, HALF * PW], dt_act,
                                 name="qs2", tag="qs2")
                for co in range(NCH):
                    ps, psv = psum_pair("ps1")
                    conv_mms2(ps, w1_sb, qa1[n], co)
                    e1 = mpool.tile([128, NCH, HALF * PW], dt.bfloat16,
                                    name="e1", tag="e1")
                    e2 = mpool.tile([128, NCH, HALF * PW], dt.bfloat16,
                                    name="e2", tag="e2")
                    nc.vector.tensor_scalar(
                        e1[:], psv, thr[:, co, 0:1], None, ALU.is_gt)
                    nc.vector.scalar_tensor_tensor(
                        e2[:], psv, thr[:, co, 1:2], e1[:], ALU.is_gt,
                        ALU.add)
                    nc.vector.scalar_tensor_tensor(
                        qs2[:, co], psv, thr[:, co, 2:3], e2[:], ALU.is_gt,
                        ALU.add)
                    # interior rows across both h-halves are uniformly
                    # 30-strided, so one 3-dim ScalarE copy lands this
                    # co's staging tile in the padded plane (engine copy
                    # beats DMA scatter for 28-byte strided rows; ScalarE
                    # has slack and Pool copies measure 2.8us)
                    nc.scalar.copy(
                        plane_interior(qa2[n], co),
                        qs2[:, co].rearrange(
                            "p g (r c) -> p (g r) c", c=PW)[:, :, 0:W])

            # conv2 -> bn2 -> +residual -> relu -> out:
            #   v = ps*s2 + x (DVE STT), out = relu(v + bb2) (ACT bias)
            def conv2_image(n):
                for co in range(NCH):
                    ps, psv = psum_pair("ps2")
                    conv_mms2(ps, w2_sb, qa2[n], co)
                    v = upool.tile([128, H, W], dt.float32, name="v", tag="v")
                    o = opool.tile([128, H, W], dt.float32, name="o", tag="o")
                    for h in range(2):
                        psvh = psv[:, h].rearrange(
                            "p (r c) -> p r c", c=PW)[:, :, 0:W]
                        nc.vector.scalar_tensor_tensor(
                            v[:][:, h * HALF:(h + 1) * HALF, :], psvh,
                            s2_sb[:, co:co + 1],
                            x_sb[n][co][:, h * HALF:(h + 1) * HALF, :],
                            ALU.mult, ALU.add)
                    nc.scalar.activation(
                        o[:], v[:], mybir.ActivationFunctionType.Relu,
                        bias=b2_sb[:, co:co + 1])
                    nc.sync.dma_start(
                        y_d[n, co * 128:(co + 1) * 128, :, :], o[:])

            # software-pipelined emission, x-quant running two images
            # ahead: on each in-order queue, instructions are emitted in
            # the order their dependencies become ready, so nothing
            # head-of-line blocks. The qa1 scatter of image n+2 is emitted
            # after conv2(n-1)'s stores, matching readiness order on sync.
            # image 0+1 x loads lead the sync ring, then the tiny Sign
            # bias constant (the image-1 quant chain blocks on it), then
            # plane fills and weights
            load_x(0)
            if NP_ > 1:
                load_x(1)
            nc.sync.dma_start(xqtn_sb[:], xqtn_d[:])
            fill_planes(0)
            if NP_ > 1:
                fill_planes(1)
            nc.sync.dma_start(w1_sb[:], w1_d[:])
            for i in range(NWARM):
                wps = pspool.tile([128, NCH, PSH], dt.float32, name="wps",
                                  tag="ps")
                nc.tensor.matmul(
                    wps[:, 0, 0:RUN],
                    wscr[:, 0:NCH, 0:128],
                    wscr[:, 0:NCH, 0:RUN],
                    start=True, stop=True,
                    perf_mode=mybir.MatmulPerfMode.DoubleRow,
                )
            xq_compute(0)
            nc.sync.dma_start(t1_sb[:], t1_d[:])
            nc.sync.dma_start(t1s_sb[:], t1s_d[:])
            nc.sync.dma_start(w2_sb[:], w2_d[:])
            nc.sync.dma_start(s2_sb[:], s2_d[:])
            nc.sync.dma_start(b2_sb[:], b2_d[:])
            if NP_ > 1:
                xq_compute(1)
            for n in range(NP_):
                conv1_image(n)
                if n + 2 < NP_:
                    xq_compute(n + 2)
                if n >= 1:
                    conv2_image(n - 1)
            conv2_image(NP_ - 1)

    nc.compile()
    return nc


_CACHED = None


def _get_program():
    global _CACHED
    if _CACHED is None:
        _CACHED = _build_program()
    return _CACHED


def kernel(x, w1, g1, b1, m1, v1, w2, g2, b2, m2, v2):
    _install_ntff_hook_shim()
    from concourse.bass_utils import run_bass_kernel_spmd

    x = np.asarray(x, np.float32)
    host = _host_arrays(w1, g1, b1, m1, v1, w2, g2, b2, m2, v2)

    xs = x.reshape(NCORES, NPER, C, H, W)
    in_maps = [{"x": np.ascontiguousarray(xs[c]), **host}
               for c in range(NCORES)]

    nc = _get_program()
    res = run_bass_kernel_spmd(
        nc, in_maps, core_ids=list(range(NCORES)),
        trace=bool(int(os.environ.get("KERNEL_TRACE", "0"))),
    )
    kernel.last_results = res
    y = np.concatenate([res.results[c]["y"][None] for c in range(NCORES)], 0)
    return np.ascontiguousarray(y.reshape(64, C, H, W).astype(np.float32))
